# revision 44
# baseline (speedup 1.0000x reference)
"""GATNet (3x GATConv graph branch + 1D-CNN protein branch + fusion MLP) on 8
Trainium2 NeuronCores via Bass/Tile.

Sharding: nodes row-sharded 1280/core (= 32 graphs/core since batch is sorted
blocks of 40); CNN branch sharded by the same 32 samples/core; weights
replicated in bf16.

Per GAT layer l:
  1. h = x @ [W | W@as_blk | W@ad_blk]  (m-outer node-block loop; attention
     scalars appear as extra columns). Augmented rows (h | a_s as f32
     bitcast | const 1) are written to local DRAM block by block.
  2. Chunked AllGather: every 2 node blocks fire their own AllGather into a
     block-major shared h_full, overlapping collectives with the remaining
     x@W compute. Gather indices are host-remapped to the chunked layout.
  3. Per 128-dst block: dst-sorted edge tiles; indirect-DMA gathers src rows
     into one batched tile; block-batched attention math (one add / Prelu /
     Exp chain over all tiles); numerator+denominator via one-hot S matmuls
     (S pre-scaled by exp for H<=2 layers, features pre-scaled for the
     10-head layer); scale by reciprocal; transpose tiles; bias+activation
     on transposed tiles -> next lhsT.

Self-contained: hardcodes all shapes; builds the per-call edge structure into
the traced program, compiles and runs via run_bass_kernel_spmd.
"""
import numpy as np
import ml_dtypes

import concourse.bass as bass
import concourse.mybir as mybir
import concourse.tile as tile
from concourse.bass_utils import run_bass_kernel_spmd
from concourse.masks import make_identity
from concourse.tile import add_dep_helper

NCORES = 8
N_NODES = 10240
N_GRAPHS = 256
NPC = N_NODES // NCORES          # 1280 nodes/core
GPC = N_GRAPHS // NCORES         # 32 graphs/core
NPG = N_NODES // N_GRAPHS        # 40 nodes/graph
BPC = NPC // 128                 # 10 dst blocks/core
# AllGather chunking per layer: lists of node-block counts per collective.
# L1's x@W is tiny, so one collective; deeper layers pipeline more chunks.
AG_CHUNKS = [[10], [4, 3, 3], [2, 2, 2, 2, 2]]
SEQ = 1000
VOCAB = 26
EMB = 128
NEG_SLOPE = 0.2

F32 = mybir.dt.float32
F32R = mybir.dt.float32r
BF16 = mybir.dt.bfloat16
I32 = mybir.dt.int32
AX = mybir.AxisListType
OP = mybir.AluOpType
ACT = mybir.ActivationFunctionType

# (F_in, F_out, heads)
LAYERS = [(78, 780, 10), (780, 1560, 2), (1560, 3120, 1)]
# x@W psum chunk lists (cover F_out + 2H cols; head-aligned where needed)
MM_CHUNKS = [[390, 390, 20], [390, 390, 390, 390, 4], [448] * 6 + [434]]
# message-pass numerator chunks: per-head col ranges for S-scaled layers
MP_HEAD_CHUNKS = {1: [(0, 512), (512, 780)],
                  2: [(0, 512), (512, 1024), (1024, 1536), (1536, 2048),
                      (2048, 2560), (2560, 3072), (3072, 3120)]}

bf = lambda a: np.ascontiguousarray(a).astype(ml_dtypes.bfloat16)
f32 = lambda a: np.ascontiguousarray(a, dtype=np.float32)
cdiv = lambda a, b: -(-a // b)


# ------------------------------------------------------------------ walrus patch
def _split_sync_waits(nc, max_keep=1):
    for f in nc.m.functions:
        for bb in f.blocks:
            out, changed = [], False
            for ins in bb.instructions:
                si = ins.sync_info
                waits = list(si.on_wait) if si is not None and si.on_wait else []
                if len(waits) > max_keep:
                    extra, keep = waits[:-max_keep], waits[-max_keep:]
                    for i in range(0, len(extra), max_keep):
                        out.append(mybir.InstNoOp(
                            name=f"WSPLIT-{nc.next_id()}", engine=ins.engine,
                            bass_nofuse=True,
                            sync_info=mybir.SyncInfo(on_wait=extra[i:i + max_keep],
                                                     on_update=[])))
                    si.on_wait = keep
                    changed = True
                out.append(ins)
            if changed:
                bb.instructions[:] = out


# ------------------------------------------------------------------ host prep
def _remap_rows(n, li):
    """Global node id -> row in layer li's chunk-major h_full layout."""
    c, j = n // NPC, n % NPC
    m = j // 128
    b0s = np.cumsum([0] + AG_CHUNKS[li][:-1])
    q = np.searchsorted(b0s, m, side="right") - 1
    b0 = b0s[q]
    nb = np.asarray(AG_CHUNKS[li])[q]
    return b0 * (NCORES * 128) + c * (nb * 128) + (j - b0 * 128)


def _edge_structure(edge_index):
    src, dst = edge_index[0].astype(np.int64), edge_index[1].astype(np.int64)
    loop = np.arange(N_NODES, dtype=np.int64)
    s_all = np.concatenate([src, loop])
    d_all = np.concatenate([dst, loop])
    order = np.argsort(d_all, kind="stable")
    s_s, d_s = s_all[order], d_all[order]

    bounds = np.searchsorted(d_s, np.arange(0, N_NODES + 1, 128))
    cnt = bounds[1:] - bounds[:-1]
    tiles_needed = -(-cnt // 128)
    T_blocks = [int(tiles_needed.reshape(NCORES, BPC)[:, p].max()) for p in range(BPC)]
    t_off = np.cumsum([0] + T_blocks)
    T_tot = int(t_off[-1])

    src_idx = np.zeros((NCORES, 3, T_tot, 128), np.int32)
    S = np.zeros((NCORES, T_tot, 128, 128), np.float32)
    for c in range(NCORES):
        for p_ in range(BPC):
            blk = c * BPC + p_
            e0, e1 = int(bounds[blk]), int(bounds[blk + 1])
            m = e1 - e0
            ti = np.arange(m) // 128 + t_off[p_]
            ei = np.arange(m) % 128
            for li in range(3):
                src_idx[c, li, ti, ei] = _remap_rows(s_s[e0:e1], li)
            S[c, ti, ei, d_s[e0:e1] - 128 * blk] = 1.0
    ST = np.swapaxes(S, 2, 3)
    SS = np.concatenate([S, ST], axis=3)          # [8, T, 128, 256]
    # [8, 128, 3*T_tot]
    src_idxT = np.ascontiguousarray(
        np.swapaxes(src_idx.reshape(NCORES, 3 * T_tot, 128), 1, 2))
    return T_blocks, src_idxT, bf(SS)


def _aug_w(W, a_s, a_d, H):
    """[W | W@as_blk | W@ad_blk] with as_blk[f,h] = a_s[h, f - h*FH]."""
    fi, fo = W.shape
    FH = fo // H
    was = np.zeros((fi, H), np.float32)
    wad = np.zeros((fi, H), np.float32)
    for h in range(H):
        was[:, h] = W[:, h * FH:(h + 1) * FH] @ a_s[h]
        wad[:, h] = W[:, h * FH:(h + 1) * FH] @ a_d[h]
    return np.concatenate([W, was, wad], axis=1)


def _bias_colmajor(b, fo):
    n_t = cdiv(fo, 128)
    pad = np.zeros(n_t * 128, np.float32)
    pad[:fo] = b
    return np.ascontiguousarray(pad.reshape(n_t, 128).T)   # [128, n_t]


def _host_prep(inputs):
    ii = {k: np.asarray(v) for k, v in inputs.items()}
    T_blocks, src_idxT, SS = _edge_structure(ii["edge_index"])

    xT = np.ascontiguousarray(np.swapaxes(f32(ii["x"]), 0, 1))   # [78, 10240]

    W_aug, b_col = [], []
    for i, (fi, fo, H) in enumerate(LAYERS):
        W_aug.append(bf(_aug_w(f32(ii[f"W{i+1}"]), f32(ii[f"as{i+1}"]),
                               f32(ii[f"ad{i+1}"]), H)))
        b_col.append(_bias_colmajor(f32(ii[f"b{i+1}"]).reshape(-1), fo))

    cw1 = f32(ii["cw1"])
    cw1f = np.zeros((125, 8, 2, 128), np.float32)
    for sc in range(8):
        for ks in range(2):
            blk = cw1[:, sc * 125:(sc + 1) * 125, ks * 4:(ks + 1) * 4]
            cw1f[:, sc, ks, :] = blk.transpose(1, 2, 0).reshape(125, 128)
    cwT = lambda w: np.ascontiguousarray(np.transpose(f32(ii[w]), (1, 2, 0)))

    w1xt = np.ascontiguousarray(
        f32(ii["fc1_xt_w"]).reshape(128, 33, 1024).transpose(1, 0, 2))

    emb_f = f32(ii["emb_xt"])                                   # [26, 128]
    rep = lambda a, n: np.ascontiguousarray(
        np.broadcast_to(f32(a).reshape(1, -1), (n, f32(a).size)))

    shared = {
        "W1": W_aug[0], "W2": W_aug[1], "W3": W_aug[2],
        "bc1": b_col[0], "bc2": b_col[1], "bc3": b_col[2],
        "fc_g1_w": bf(ii["fc_g1_w"]), "fc_g1_b": rep(ii["fc_g1_b"], GPC),
        "fc_g2_w": f32(ii["fc_g2_w"]), "fc_g2_b": rep(ii["fc_g2_b"], GPC),
        "cw1f": bf(cw1f), "cb1": f32(ii["cb1"]).reshape(-1, 1),
        "cw2T": bf(cwT("cw2")), "cb2": f32(ii["cb2"]).reshape(-1, 1),
        "cw3T": bf(cwT("cw3")), "cb3": f32(ii["cb3"]).reshape(-1, 1),
        "cw4T": bf(cwT("cw4")), "cb4": f32(ii["cb4"]).reshape(-1, 1),
        "w1xt": bf(w1xt), "fc1_xt_b": rep(ii["fc1_xt_b"], GPC),
        "fc2_xt_w": f32(ii["fc2_xt_w"]), "fc2_xt_b": rep(ii["fc2_xt_b"], GPC),
        "fc1_w": f32(ii["fc1_w"]), "fc1_b": rep(ii["fc1_b"], GPC),
        "fc2_w": f32(ii["fc2_w"]), "fc2_b": rep(ii["fc2_b"], GPC),
        "out_w": f32(ii["out_w"]),
    }
    in_maps = []
    for c in range(NCORES):
        m = dict(shared)
        m["xT"] = bf(xT[:, c * NPC:(c + 1) * NPC])
        m["esrcT"] = src_idxT[c]
        m["SS"] = SS[c]
        tgt_c = ii["target"][c * GPC:(c + 1) * GPC]               # [32, 1000]
        E_full = emb_f[tgt_c]                                     # [32, 1000, 128]
        E2 = E_full.reshape(8, 4, 8, 125, 128).transpose(2, 0, 3, 1, 4)
        m["E2"] = bf(E2.reshape(8, 8, 125, 512))
        in_maps.append(m)
    out_b = float(np.asarray(ii["out_b"]).reshape(-1)[0])
    return T_blocks, in_maps, out_b


# ------------------------------------------------------------------ program
class P:
    pass


def _aug_cols(li):
    fo, H = LAYERS[li][1], LAYERS[li][2]
    return fo + 2 * H + 2        # h | a_s(f32 as 2H bf16) | ones | pad


def build_program(T_blocks, taps=()):
    T_tot = sum(T_blocks)
    nc = bass.Bass()
    p = P()
    p.nc = nc
    p.taps = set(taps)
    p.tap_tensors = {}

    dp = lambda name, shape, dt: nc.declare_dram_parameter(name, list(shape), dt,
                                                           isOutput=False)
    p.xT = dp("xT", [78, NPC], BF16)
    p.W = [dp(f"W{i+1}", [LAYERS[i][0], LAYERS[i][1] + 2 * LAYERS[i][2]], BF16)
           for i in range(3)]
    p.bc = [dp(f"bc{i+1}", [128, cdiv(LAYERS[i][1], 128)], F32) for i in range(3)]
    p.esrcT = dp("esrcT", [128, 3 * T_tot], I32)
    p.SS = dp("SS", [T_tot, 128, 256], BF16)
    p.fc_g1_w = dp("fc_g1_w", [3120, 1024], BF16)
    p.fc_g1_b = dp("fc_g1_b", [GPC, 1024], F32)
    p.fc_g2_w = dp("fc_g2_w", [1024, 128], F32)
    p.fc_g2_b = dp("fc_g2_b", [GPC, 128], F32)
    p.E2 = dp("E2", [8, 8, 125, 512], BF16)
    p.cw1f = dp("cw1f", [125, 8, 2, 128], BF16)
    p.cb1 = dp("cb1", [32, 1], F32)
    p.cw2T = dp("cw2T", [32, 8, 64], BF16)
    p.cb2 = dp("cb2", [64, 1], F32)
    p.cw3T = dp("cw3T", [64, 8, 96], BF16)
    p.cb3 = dp("cb3", [96, 1], F32)
    p.cw4T = dp("cw4T", [96, 8, 128], BF16)
    p.cb4 = dp("cb4", [128, 1], F32)
    p.w1xt = dp("w1xt", [33, 128, 1024], BF16)
    p.fc1_xt_b = dp("fc1_xt_b", [GPC, 1024], F32)
    p.fc2_xt_w = dp("fc2_xt_w", [1024, 128], F32)
    p.fc2_xt_b = dp("fc2_xt_b", [GPC, 128], F32)
    p.fc1_w = dp("fc1_w", [256, 1024], F32)
    p.fc1_b = dp("fc1_b", [GPC, 1024], F32)
    p.fc2_w = dp("fc2_w", [1024, 256], F32)
    p.fc2_b = dp("fc2_b", [GPC, 256], F32)
    p.out_w = dp("out_w", [256, 1], F32)
    p.out = nc.declare_dram_parameter("out", [GPC, 1], F32, isOutput=True)

    p.h_loc = [nc.dram_tensor(f"h{i+1}_loc", [NPC, _aug_cols(i)], BF16)
               for i in range(3)]
    p.h_full = [nc.dram_tensor(f"h{i+1}_full", [N_NODES, _aug_cols(i)], BF16,
                               addr_space="Shared") for i in range(3)]

    def tap(name, shape, dt=F32):
        if name in p.taps:
            t = nc.declare_dram_parameter("tap_" + name, list(shape), dt,
                                          isOutput=True)
            p.tap_tensors[name] = t
            return t
        return None

    with tile.TileContext(nc) as tc:
        p.tc = tc
        _cp_cm = tc.tile_pool(name="const", bufs=1)
        const_pool = _cp_cm.__enter__()
        p.ident = const_pool.tile([128, 128], BF16)
        make_identity(nc, p.ident[:])
        p.ones_col = const_pool.tile([128, 1], BF16, tag="ones1", name="ones1")
        nc.vector.memset(p.ones_col[:], 1.0)
        p.join_dummy = const_pool.tile([1, 1], F32, tag="jd", name="jd")
        p.head_pool = const_pool

        stages = _cnn_make(p, tap)
        p.cnn_stages = stages
        _gat_branch(p, T_blocks, tap)
        _fusion(p, tap)
        for cm in p.gat_cleanup:
            cm.__exit__(None, None, None)
        _cp_cm.__exit__(None, None, None)

    _split_sync_waits(nc)
    return nc, p


# ---------------- GAT branch ----------------
def _gat_branch(p, T_blocks, tap):
    nc, tc = p.nc, p.tc

    mpc_cm = tc.tile_pool(name="mpc", bufs=1)
    mpc_pool = mpc_cm.__enter__()
    eidx = mpc_pool.tile([128, 3 * sum(T_blocks)], I32, tag="eidx", name="eidx")
    nc.sync.dma_start(out=eidx[:], in_=p.esrcT[:])
    p.eidx = eidx
    adp_cms = [tc.tile_pool(name=f"adp{li}", bufs=1) for li in range(3)]
    adp_pools = [cm.__enter__() for cm in adp_cms]

    xT_cm = tc.tile_pool(name="xT0", bufs=1)
    xT_pool = xT_cm.__enter__()
    xT_tiles = [xT_pool.tile([78, NPC], BF16, tag="x0", name="x0")]
    nc.sync.dma_start(out=xT_tiles[0][:], in_=p.xT[:])

    for li, (fi, fo, H) in enumerate(LAYERS):
        is_last = li == 2
        n_k = cdiv(fi, 128)
        cols = _aug_cols(li)
        a_d_pool = adp_pools[li]
        a_d_tiles = []
        chunks = MM_CHUNKS[li]
        offs = [int(v) for v in np.cumsum([0] + chunks)]
        n_ch = len(chunks)
        ag_insts = []
        with (
            tc.tile_pool(name=f"w{li}", bufs=1) as wpool,
            tc.tile_pool(name=f"mm{li}", bufs=2) as mpool,
            tc.tile_pool(name=f"mmp{li}", bufs=2 if li == 0 else 1,
                         space="PSUM") as pspool,
        ):
            W_sb = []
            for k in range(n_k):
                kp = min(128, fi - k * 128)
                t = wpool.tile([kp, offs[-1]], BF16, tag=f"W{k}", name=f"W{k}")
                nc.sync.dma_start(out=t[:], in_=p.W[li][k * 128:k * 128 + kp, :])
                W_sb.append(t)
            pending_writes = []
            for m in range(BPC):
                psums = [pspool.tile([128, chunks[n]], F32, tag=f"hp{n}",
                                     name=f"hp{n}")
                         for n in range(n_ch)]
                for k in range(n_k):
                    kp = min(128, fi - k * 128)
                    lhs = xT_tiles[k][:kp, m * 128:(m + 1) * 128]
                    for n in range(n_ch):
                        nc.tensor.matmul(
                            psums[n][:], lhs, W_sb[k][:, offs[n]:offs[n + 1]],
                            start=(k == 0), stop=(k == n_k - 1))
                stage = mpool.tile([128, cols], BF16, tag="stage", name="stage")
                for n in range(n_ch):
                    lo, hi = offs[n], offs[n + 1]
                    if hi <= fo:
                        nc.scalar.copy(out=stage[:, lo:hi], in_=psums[n][:])
                    else:
                        if lo < fo:
                            nc.scalar.copy(out=stage[:, lo:fo],
                                           in_=psums[n][:, :fo - lo])
                        nc.vector.tensor_copy(
                            out=stage[:, fo:fo + 2 * H].bitcast(F32),
                            in_=psums[n][:, fo - lo:fo - lo + H])
                        a_d = a_d_pool.tile([128, H], BF16, tag=f"a_d{m}",
                                            name=f"a_d{m}")
                        nc.vector.tensor_copy(
                            out=a_d[:],
                            in_=psums[n][:, fo - lo + H:fo - lo + 2 * H])
                        a_d_tiles.append(a_d)
                w = nc.scalar.dma_start(
                    out=p.h_loc[li][m * 128:(m + 1) * 128, :], in_=stage[:])
                pending_writes.append(w)
                b0s = [int(v) for v in np.cumsum([0] + AG_CHUNKS[li][:-1])]
                if m + 1 - b0s[len(ag_insts)] == AG_CHUNKS[li][len(ag_insts)]:
                    b0, nb = b0s[len(ag_insts)], AG_CHUNKS[li][len(ag_insts)]
                    cc = nc.gpsimd.collective_compute(
                        "AllGather", OP.bypass,
                        replica_groups=[list(range(NCORES))],
                        ins=[p.h_loc[li][b0 * 128:(b0 + nb) * 128, :]],
                        outs=[p.h_full[li][b0 * NCORES * 128:
                                           (b0 + nb) * NCORES * 128, :]])
                    for w_ in pending_writes:
                        add_dep_helper(cc.ins, w_.ins, reason="AG waits h_loc")
                    pending_writes = []
                    ag_insts.append(cc)

        # join: one gpsimd op that waits for all AG chunks of this layer
        join = nc.gpsimd.memset(p.join_dummy[:], float(li))
        for cc in ag_insts:
            add_dep_helper(join.ins, cc.ins, reason="join waits AG chunk")

        t = tap(f"h{li+1}", [NPC, cols], BF16)
        if t is not None:
            d = nc.sync.dma_start(out=t[:], in_=p.h_loc[li][:])
            add_dep_helper(d.ins, join.ins, reason="tap waits AGs")

        xT_cm.__exit__(None, None, None)

        filler = None
        if li == 0:
            p.cnn_stages["stage1_open"]()

            def filler(blk):
                if blk < 8:
                    p.cnn_stages["stage1_grp"](blk)
                elif blk == 8:
                    p.cnn_stages["stage1_close"]()
        elif li == 1:
            p.cnn_stages["stage2"]()
        elif li == 2:
            p.cnn_stages["stage3"]()

        n_kT = cdiv(fo, 128)
        xTn_cm = tc.tile_pool(name=f"xTn{li}", bufs=1)
        xTn_pool = xTn_cm.__enter__()
        xT_out = []
        for j in range(n_kT):
            kp = min(128, fo - j * 128)
            xT_out.append(xTn_pool.tile([kp, NPC], BF16, tag=f"xT{li}_{j}",
                                        name=f"xT{li}_{j}"))

        _message_pass(p, T_blocks, li, a_d_tiles, join, xT_out, filler)

        t = tap(f"xT{li+2}" if not is_last else "o3T", [fo, NPC], BF16)
        if t is not None:
            for j in range(n_kT):
                kp = min(128, fo - j * 128)
                nc.sync.dma_start(out=t[j * 128:j * 128 + kp, :], in_=xT_out[j][:])

        xT_tiles = xT_out
        xT_cm = xTn_cm
        if is_last:
            p.out3T = xT_out
            p.gat_cleanup = [xTn_cm, p.cnn_stages["cleanup_cm"],
                             *reversed(adp_cms), mpc_cm]
    return


def _message_pass(p, T_blocks, li, a_d_tiles, ag_join, xT_out, filler=None):
    nc, tc = p.nc, p.tc
    fi, fo, H = LAYERS[li]
    FH = fo // H
    cols = _aug_cols(li)
    t_off = np.cumsum([0] + T_blocks)
    n_kT = cdiv(fo, 128)
    s_scale = li > 0                 # scale S by exp for H<=2; scale g for H=10
    nsub = 1 if li == 0 else 2

    with (
        tc.tile_pool(name=f"mp{li}", bufs=2) as mp,
        tc.tile_pool(name=f"mpS{li}", bufs=2) as mpS,
        tc.tile_pool(name=f"bc{li}", bufs=1) as bcp,
    ):
        bcol = bcp.tile([128, n_kT], F32, tag="bcol", name="bcol")
        nc.sync.dma_start(out=bcol[:], in_=p.bc[li][:])

        for blk in range(BPC):
            if filler is not None:
                filler(blk)
            Tb = T_blocks[blk]
            t0 = int(t_off[blk])
            e0 = li * int(t_off[-1]) + t0
            subs = [(0, Tb)] if nsub == 1 else \
                [(0, (Tb + 1) // 2), ((Tb + 1) // 2, Tb - (Tb + 1) // 2)]
            rows_bf = mp.tile([128, fo], BF16, tag="rows_bf", name="rows_bf")
            rec = mp.tile([128, H], F32, tag="rec", name="rec")
            with (
                tc.tile_pool(name=f"op{li}_{blk}", bufs=1, space="PSUM") as pp,
            ):
                if s_scale:
                    head_chunks = MP_HEAD_CHUNKS[li]
                    n_hc = len(head_chunks)
                    dlast = head_chunks[-1][1] - head_chunks[-1][0]
                    opsum = {}
                    for h in range(H):
                        for ci, (lo, hi) in enumerate(head_chunks):
                            w_ = hi - lo + (1 if ci == n_hc - 1 else 0)
                            opsum[(h, ci)] = pp.tile(
                                [128, w_], F32,
                                tag=f"op{h}_{ci}", name=f"op{h}_{ci}")
                else:
                    num0 = pp.tile([128, 390], F32, tag="num0", name="num0")
                    num1 = pp.tile([128, 390], F32, tag="num1", name="num1")
                    dn = pp.tile([128, H], F32, tag="dn", name="dn")
                aux = pp.tile([128, Tb * H], F32, tag="aux", name="aux")

                for si, (ta, tn) in enumerate(subs):
                    is_fs, is_ls = si == 0, si == nsub - 1
                    SS_sub = mpS.tile([128, tn, 256], BF16, tag=f"SS{si}",
                                      name=f"SS{si}")
                    nc.sync.dma_start(
                        out=SS_sub[:],
                        in_=p.SS[t0 + ta:t0 + ta + tn].rearrange(
                            "t p c -> p t c"))
                    g_sub = mp.tile([128, tn, cols], BF16, tag=f"g{si}",
                                    name=f"g{si}", bufs=3)
                    for t in range(tn):
                        gi = nc.gpsimd.indirect_dma_start(
                            out=g_sub[:, t, :], out_offset=None,
                            in_=p.h_full[li][:],
                            in_offset=bass.IndirectOffsetOnAxis(
                                ap=p.eidx[:, e0 + ta + t:e0 + ta + t + 1],
                                axis=0))
                        add_dep_helper(gi.ins, ag_join.ins,
                                       reason="gather waits AG")
                    for t in range(tn):
                        nc.tensor.matmul(
                            aux[:, (ta + t) * H:(ta + t + 1) * H],
                            SS_sub[:, t, 128:256],
                            a_d_tiles[blk][:], start=True, stop=True)
                    sc = mp.tile([128, tn * H], F32, tag=f"sc{si}",
                                 name=f"sc{si}")
                    nc.vector.tensor_tensor(
                        out=sc[:].rearrange("p (t h) -> p t h", h=H),
                        in0=g_sub[:, :, fo:fo + 2 * H].bitcast(F32),
                        in1=aux[:, ta * H:(ta + tn) * H].rearrange(
                            "p (t h) -> p t h", h=H),
                        op=OP.add)
                    nc.scalar.activation(sc[:], sc[:], ACT.Prelu,
                                         alpha=NEG_SLOPE)
                    ex = mp.tile([128, tn * H], BF16, tag=f"ex{si}",
                                 name=f"ex{si}")
                    nc.scalar.activation(ex[:], sc[:], ACT.Exp)
                    ex3 = ex[:].rearrange("p (t h) -> p t h", h=H)

                    if s_scale:
                        Ssc = []
                        for h in range(H):
                            sh = mp.tile([128, tn * 128], BF16,
                                         tag=f"Ssc{si}_{h}", name=f"Ssc{si}_{h}")
                            nc.vector.tensor_tensor(
                                out=sh[:].rearrange("p (t c) -> p t c", c=128),
                                in0=SS_sub[:, :, 0:128],
                                in1=ex3[:, :, h:h + 1].broadcast_to(
                                    [128, tn, 128]),
                                op=OP.mult)
                            Ssc.append(sh)
                        # denominators first (own their bank's initial clear)
                        for h in range(H):
                            for t in range(tn):
                                nc.tensor.matmul(
                                    opsum[(h, n_hc - 1)][:, dlast:dlast + 1],
                                    Ssc[h][:, t * 128:(t + 1) * 128],
                                    p.ones_col[:],
                                    start=(is_fs and t == 0),
                                    stop=(is_ls and t == tn - 1),
                                    skip_group_check=not is_fs)
                        if is_ls:
                            for h in range(H):
                                nc.vector.tensor_scalar(
                                    out=rec[:, h:h + 1],
                                    in0=opsum[(h, n_hc - 1)][:,
                                              dlast:dlast + 1],
                                    scalar1=1e-16, scalar2=None, op0=OP.add)
                            nc.vector.reciprocal(rec[:], rec[:])
                        # numerators chunk-major within this sub-unit; the
                        # last chunk's bank was cleared by the denominator
                        # group -> start=False overwrite-on-cleared
                        for ci, (lo, hi) in enumerate(head_chunks):
                            is_dl = ci == n_hc - 1
                            for h in range(H):
                                for t in range(tn):
                                    nc.tensor.matmul(
                                        opsum[(h, ci)][:, :hi - lo],
                                        Ssc[h][:, t * 128:(t + 1) * 128],
                                        g_sub[:, t, h * FH + lo:h * FH + hi],
                                        start=(is_fs and t == 0 and not is_dl),
                                        stop=(is_ls and t == tn - 1),
                                        skip_group_check=is_dl or not is_fs)
                                if is_ls:
                                    nc.vector.tensor_scalar(
                                        out=rows_bf[:,
                                                    h * FH + lo:h * FH + hi],
                                        in0=opsum[(h, ci)][:, :hi - lo],
                                        scalar1=rec[:, h:h + 1], scalar2=None,
                                        op0=OP.mult)
                    else:
                        # one 4D broadcast multiply scales all heads at once
                        gs = mp.tile([128, tn, fo + H], BF16, tag=f"gs{si}",
                                     name=f"gs{si}", bufs=3)
                        nc.vector.tensor_tensor(
                            out=gs[:, :, 0:fo].rearrange(
                                "p t (h f) -> p t h f", f=FH),
                            in0=g_sub[:, :, 0:fo].rearrange(
                                "p t (h f) -> p t h f", f=FH),
                            in1=ex3.unsqueeze(3).broadcast_to(
                                [128, tn, H, FH]),
                            op=OP.mult)
                        nc.vector.tensor_copy(out=gs[:, :, fo:fo + H], in_=ex3)
                        for t in range(tn):
                            nc.tensor.matmul(
                                dn[:], SS_sub[:, t, 0:128],
                                gs[:, t, fo:fo + H],
                                start=(is_fs and t == 0),
                                stop=(is_ls and t == tn - 1))
                        if is_ls:
                            nc.vector.tensor_scalar(
                                out=rec[:], in0=dn[:], scalar1=1e-16,
                                scalar2=None, op0=OP.add)
                            nc.vector.reciprocal(rec[:], rec[:])
                        for ni, (nt, lo, hi) in enumerate(
                                [(None, 0, 390), (None, 390, 780)]):
                            tgt = num0 if ni == 0 else num1
                            for t in range(tn):
                                nc.tensor.matmul(
                                    tgt[:], SS_sub[:, t, 0:128],
                                    gs[:, t, lo:hi],
                                    start=(is_fs and t == 0),
                                    stop=(is_ls and t == tn - 1))
                            if is_ls:
                                nh = 5
                                h0 = 0 if ni == 0 else 5
                                nc.vector.tensor_tensor(
                                    out=rows_bf[:, lo:hi].rearrange(
                                        "p (h f) -> p h f", f=FH),
                                    in0=tgt[:].rearrange(
                                        "p (h f) -> p h f", f=FH),
                                    in1=rec[:, h0:h0 + nh].unsqueeze(
                                        2).broadcast_to([128, nh, FH]),
                                    op=OP.mult)
            if s_scale:
                with tc.tile_pool(name=f"tp{li}_{blk}", bufs=2,
                                  space="PSUM") as ptp:
                    for j in range(n_kT):
                        kp = min(128, fo - j * 128)
                        tp = ptp.tile([kp, 128], BF16, tag="tp", name="tp")
                        nc.tensor.transpose(
                            tp[:], rows_bf[:, j * 128:j * 128 + kp], p.ident[:])
                        nc.scalar.activation(
                            xT_out[j][:, blk * 128:(blk + 1) * 128], tp[:],
                            ACT.Relu, bias=bcol[:kp, j:j + 1])
            else:
                # transpose + batched ELU epilogue
                with tc.tile_pool(name=f"tp{li}_{blk}", bufs=2,
                                  space="PSUM") as ptp:
                    zall = mp.tile([128, n_kT * 128], F32, tag="zall",
                                   name="zall")
                    for j in range(n_kT):
                        kp = min(128, fo - j * 128)
                        tp = ptp.tile([kp, 128], BF16, tag="tp", name="tp")
                        nc.tensor.transpose(
                            tp[:], rows_bf[:, j * 128:j * 128 + kp], p.ident[:])
                        nc.scalar.activation(
                            zall[:kp, j * 128:(j + 1) * 128], tp[:],
                            ACT.Identity, bias=bcol[:kp, j:j + 1])
                    t1 = mp.tile([128, n_kT * 128], F32, tag="elu1", name="elu1")
                    nc.vector.tensor_scalar(out=t1[:], in0=zall[:], scalar1=0.0,
                                            scalar2=None, op0=OP.min)
                    nc.scalar.activation(t1[:], t1[:], ACT.Exp)
                    nc.scalar.activation(zall[:], zall[:], ACT.Relu)
                    for j in range(n_kT):
                        kp = min(128, fo - j * 128)
                        nc.vector.scalar_tensor_tensor(
                            out=xT_out[j][:, blk * 128:(blk + 1) * 128],
                            in0=zall[:kp, j * 128:(j + 1) * 128], scalar=-1.0,
                            in1=t1[:kp, j * 128:(j + 1) * 128],
                            op0=OP.add, op1=OP.add)


def _dve_T(nc, dst, src, n):
    """dst[n, 32] = src[32, n].T via DVE 32x32 block transposes."""
    for i in range(n // 32):
        nc.vector.transpose(out=dst[32 * i:32 * (i + 1), :],
                            in_=src[:, 32 * i:32 * (i + 1)])


# ---------------- graph head ----------------
def _graph_head(p, tap):
    nc, tc = p.nc, p.tc
    n_kT = len(p.out3T)
    with (
        tc.tile_pool(name="gh", bufs=2) as gh,
        tc.tile_pool(name="ghG", bufs=1) as ghG,
        tc.tile_pool(name="ghp", bufs=2, space="PSUM") as ghp,
    ):
        gT = [ghG.tile([min(128, 3120 - j * 128), GPC], BF16, tag=f"gT{j}", name=f"gT{j}")
              for j in range(n_kT)]
        for j in range(n_kT):
            kp = min(128, 3120 - j * 128)
            gm = gh.tile([kp, GPC * 20], BF16, tag="gmx", name="gmx")
            v = p.out3T[j][:].rearrange("p (g n) -> p g n", n=NPG)
            nc.vector.tensor_tensor(
                out=gm[:].rearrange("p (g n) -> p g n", n=20),
                in0=v[:, :, 0:20], in1=v[:, :, 20:40], op=OP.max)
            nc.vector.reduce_max(
                gT[j][:], gm[:].rearrange("p (g n) -> p g n", n=20),
                axis=AX.X)
        g1 = ghG.tile([GPC, 1024], F32, tag="g1", name="g1")
        psn = [ghp.tile([GPC, 512], F32, tag=f"mm{n}", name=f"mm{n}", bufs=1)
               for n in range(2)]
        for j in range(n_kT):
            kp = min(128, 3120 - j * 128)
            w = gh.tile([kp, 1024], BF16, tag="fg1w", name="fg1w", bufs=3)
            nc.sync.dma_start(out=w[:], in_=p.fc_g1_w[j * 128:j * 128 + kp, :])
            for n in range(2):
                nc.tensor.matmul(psn[n][:], gT[j][:],
                                 w[:, n * 512:(n + 1) * 512], start=(j == 0),
                                 stop=(j == n_kT - 1))
        for n in range(2):
            nc.vector.tensor_copy(out=g1[:, n * 512:(n + 1) * 512],
                                  in_=psn[n][:])
        bb1 = gh.tile([GPC, 1024], F32, tag="ghbb", name="ghbb")
        nc.sync.dma_start(out=bb1[:], in_=p.fc_g1_b[:])
        nc.vector.tensor_tensor(out=g1[:], in0=g1[:], in1=bb1[:], op=OP.add)
        g1b = ghG.tile([GPC, 1024], F32, tag="g1b", name="g1b")
        nc.scalar.activation(g1b[:], g1[:], ACT.Relu)
        g1T = [ghG.tile([128, GPC], F32, tag=f"g1T{j}", name=f"g1T{j}") for j in range(8)]
        for j in range(8):
            _dve_T(nc, g1T[j], g1b[:, j * 128:(j + 1) * 128], 128)
        ps = ghp.tile([GPC, 128], F32, tag="mm", name="mm")
        w8 = gh.tile([128, 8, 128], F32, tag="fg2w", name="fg2w")
        nc.sync.dma_start(out=w8[:], in_=p.fc_g2_w[:].rearrange(
            "(j p) n -> p j n", p=128))
        for j in range(8):
            nc.tensor.matmul(ps[:], g1T[j][:], w8[:, j, :], start=(j == 0),
                             stop=(j == 7))
        p.g2 = p.head_pool.tile([GPC, 128], F32, tag="g2", name="g2")
        bb2 = gh.tile([GPC, 128], F32, tag="ghbb2", name="ghbb2")
        nc.sync.dma_start(out=bb2[:], in_=p.fc_g2_b[:])
        nc.vector.tensor_tensor(out=p.g2[:], in0=ps[:], in1=bb2[:], op=OP.add)
        t = tap("g2", [GPC, 128])
        if t is not None:
            nc.sync.dma_start(out=t[:], in_=p.g2[:])


# ---------------- CNN branch ----------------
def _cnn_make(p, tap):
    """CNN branch split into stages so the orchestrator can interleave them
    into the AllGather gaps. Pools open at stage1, closed via cleanup_cm."""
    nc, tc = p.nc, p.tc
    st = {}

    class _Cleanup:
        def __exit__(self, *a):
            for cm in st["cms"]:
                cm.__exit__(None, None, None)

    def stage1_open():
        cn_cm = tc.tile_pool(name="cn", bufs=3)
        cnw_cm = tc.tile_pool(name="cnw", bufs=1)
        cny_cm = tc.tile_pool(name="cny", bufs=1)
        cn = cn_cm.__enter__()
        cnw = cnw_cm.__enter__()
        cny = cny_cm.__enter__()
        st["cms"] = [cny_cm, cnw_cm, cn_cm]
        st["cn"], st["cnw"], st["cny"] = cn, cnw, cny

        cw1f_sb = cny.tile([125, 8, 2, 128], BF16, tag="cw1f", name="cw1f")
        nc.sync.dma_start(out=cw1f_sb[:], in_=p.cw1f[:])
        cw2_sb = cnw.tile([32, 8, 64], BF16, tag="cw2", name="cw2")
        nc.sync.dma_start(out=cw2_sb[:], in_=p.cw2T[:])
        cw3_sb = cnw.tile([64, 8, 96], BF16, tag="cw3", name="cw3")
        nc.sync.dma_start(out=cw3_sb[:], in_=p.cw3T[:])
        cw4_sb = cnw.tile([96, 8, 128], BF16, tag="cw4", name="cw4")
        nc.sync.dma_start(out=cw4_sb[:], in_=p.cw4T[:])
        cb = {}
        for nm, sh in [("cb1", 32), ("cb2", 64), ("cb3", 96), ("cb4", 128)]:
            cb[nm] = cnw.tile([sh, 1], F32, tag=nm, name=nm)
            nc.sync.dma_start(out=cb[nm][:], in_=getattr(p, nm)[:])
        st.update(cw1f=cw1f_sb, cw2=cw2_sb, cw3=cw3_sb, cw4=cw4_sb, cb=cb)
        st["y1"] = cny.tile([32, GPC * 121], BF16, tag="y1", name="y1")
        st["cnp1_cm"] = tc.tile_pool(name="cnp1", bufs=2, space="PSUM")
        st["cnp1"] = st["cnp1_cm"].__enter__()

    def stage1_grp(grp):
        cn, cnp, cb = st["cn"], st["cnp1"], st["cb"]
        cw1f_sb, y1 = st["cw1f"], st["y1"]
        pc = [cnp.tile([128, 512], F32, tag=f"pc{k}", name=f"pc{k}", bufs=1)
              for k in range(2)]
        for sc in range(8):
            E = cn.tile([125, 512], BF16, tag="E", name="E")
            nc.sync.dma_start(out=E[:], in_=p.E2[sc, grp])
            for ks in range(2):
                nc.tensor.matmul(pc[ks][:], cw1f_sb[:, sc, ks, :], E[:],
                                 start=(sc == 0), stop=(sc == 7))
        acc = cn.tile([32, 4 * 121], F32, tag="c1acc", name="c1acc")
        accr = acc[:].rearrange("p (b t) -> p b t", b=4)
        firstop = True
        for ks in range(2):
            for kl in range(4):
                k = ks * 4 + kl
                src = pc[ks][:].rearrange("p (b j) -> p b j", b=4)[
                    kl * 32:(kl + 1) * 32, :, k:k + 121]
                if firstop:
                    nc.vector.tensor_copy(out=accr, in_=src)
                    firstop = False
                else:
                    nc.vector.tensor_tensor(out=accr, in0=accr, in1=src,
                                            op=OP.add)
        nc.scalar.activation(y1[:, grp * 4 * 121:(grp + 1) * 4 * 121],
                             acc[:], ACT.Relu, bias=cb["cb1"][:32, :1])

    def stage1_close():
        st["cnp1_cm"].__exit__(None, None, None)

    def stage2():
        cn, cny, cb = st["cn"], st["cny"], st["cb"]
        cw2_sb, cw3_sb, cw4_sb = st["cw2"], st["cw3"], st["cw4"]
        y1 = st["y1"]
        with tc.tile_pool(name="cnp2", bufs=2, space="PSUM") as cnp:
            y2 = cny.tile([64, GPC * 114], BF16, tag="y2", name="y2")
            for grp in range(8):
                ps = cnp.tile([64, 4 * 114], F32, tag="pc0", name="pc0")
                for k in range(8):
                    rhs = y1[:].rearrange("p (b t) -> p b t", t=121)[
                        :, grp * 4:(grp + 1) * 4, k:k + 114]
                    nc.tensor.matmul(ps[:], cw2_sb[:, k, :], rhs, start=(k == 0),
                                     stop=(k == 7))
                nc.scalar.activation(y2[:, grp * 4 * 114:(grp + 1) * 4 * 114], ps[:],
                                     ACT.Relu, bias=cb["cb2"][:, :1])
            y3 = cny.tile([96, GPC * 107], BF16, tag="y3", name="y3")
            for grp in range(8):
                ps = cnp.tile([96, 4 * 107], F32, tag="pc0", name="pc0")
                for k in range(8):
                    rhs = y2[:].rearrange("p (b t) -> p b t", t=114)[
                        :, grp * 4:(grp + 1) * 4, k:k + 107]
                    nc.tensor.matmul(ps[:], cw3_sb[:, k, :], rhs, start=(k == 0),
                                     stop=(k == 7))
                nc.scalar.activation(y3[:, grp * 4 * 107:(grp + 1) * 4 * 107], ps[:],
                                     ACT.Relu, bias=cb["cb3"][:, :1])
            yp = cny.tile([128, GPC * 33], BF16, tag="yp", name="yp")
            st["yp"] = yp
            for grp in range(8):
                ps = cnp.tile([128, 4 * 100], F32, tag="pc0", name="pc0")
                for k in range(8):
                    rhs = y3[:].rearrange("p (b t) -> p b t", t=107)[
                        :, grp * 4:(grp + 1) * 4, k:k + 100]
                    nc.tensor.matmul(ps[:], cw4_sb[:, k, :], rhs, start=(k == 0),
                                     stop=(k == 7))
                psr = ps[:].rearrange("p (b t) -> p b t", b=4)
                mx = cn.tile([128, 4 * 33], F32, tag="mx", name="mx")
                mxr = mx[:].rearrange("p (b t) -> p b t", b=4)
                nc.vector.tensor_copy(out=mxr, in_=psr[:, :, 0:99:3])
                nc.vector.tensor_tensor(out=mxr, in0=mxr, in1=psr[:, :, 1:100:3],
                                        op=OP.max)
                nc.vector.tensor_tensor(out=mxr, in0=mxr, in1=psr[:, :, 2:100:3],
                                        op=OP.max)
                nc.scalar.activation(yp[:, grp * 4 * 33:(grp + 1) * 4 * 33], mx[:],
                                     ACT.Relu, bias=cb["cb4"][:, :1])

    def stage3():
        cn, cny = st["cn"], st["cny"]
        yp = st["yp"]
        with tc.tile_pool(name="cnp3", bufs=2, space="PSUM") as cnp:
            xt1 = cny.tile([GPC, 1024], F32, tag="xt1", name="xt1")
            psn = [cnp.tile([GPC, 512], F32, tag=f"pc0_{n}", name=f"pc0_{n}",
                            bufs=1) for n in range(2)]
            for tg in range(9):
                t0_, t1_ = tg * 4, min(tg * 4 + 4, 33)
                w = cny.tile([128, t1_ - t0_, 1024], BF16, tag="fx1w",
                             name="fx1w", bufs=3)
                nc.sync.dma_start(out=w[:], in_=p.w1xt[t0_:t1_].rearrange(
                    "t p n -> p t n"))
                for t_ in range(t0_, t1_):
                    lhs = yp[:].rearrange("p (b t) -> p t b", t=33)[:, t_, :]
                    for n in range(2):
                        nc.tensor.matmul(
                            psn[n][:], lhs,
                            w[:, t_ - t0_, n * 512:(n + 1) * 512],
                            start=(t_ == 0), stop=(t_ == 32))
            for n in range(2):
                nc.vector.tensor_copy(out=xt1[:, n * 512:(n + 1) * 512],
                                      in_=psn[n][:])
            bb = cn.tile([GPC, 1024], F32, tag="fxbb", name="fxbb", bufs=1)
            nc.sync.dma_start(out=bb[:], in_=p.fc1_xt_b[:])
            nc.vector.tensor_tensor(out=xt1[:], in0=xt1[:], in1=bb[:], op=OP.add)
            nc.scalar.activation(xt1[:], xt1[:], ACT.Relu)
            xt1T = [cny.tile([128, GPC], F32, tag=f"xt1T{j}", name=f"xt1T{j}",
                             bufs=1)
                    for j in range(8)]
            for j in range(8):
                _dve_T(nc, xt1T[j], xt1[:, j * 128:(j + 1) * 128], 128)
            ps = cnp.tile([GPC, 128], F32, tag="pc0", name="pc0")
            w8 = cny.tile([128, 8, 128], F32, tag="fx2w", name="fx2w", bufs=1)
            nc.sync.dma_start(out=w8[:], in_=p.fc2_xt_w[:].rearrange(
                "(j p) n -> p j n", p=128))
            for j in range(8):
                nc.tensor.matmul(ps[:], xt1T[j][:], w8[:, j, :], start=(j == 0),
                                 stop=(j == 7))
            p.xt2 = p.head_pool.tile([GPC, 128], F32, tag="xt2", name="xt2")
            bb2 = cn.tile([GPC, 128], F32, tag="fxbb2", name="fxbb2", bufs=1)
            nc.sync.dma_start(out=bb2[:], in_=p.fc2_xt_b[:])
            nc.vector.tensor_tensor(out=p.xt2[:], in0=ps[:], in1=bb2[:], op=OP.add)
            t = tap("xt2", [GPC, 128])
            if t is not None:
                nc.sync.dma_start(out=t[:], in_=p.xt2[:])
        # whole CNN branch done -- release all its pools (LIFO: cny, cnw, cn)
        while st["cms"]:
            st["cms"].pop(0).__exit__(None, None, None)

    return {"stage1_open": stage1_open, "stage1_grp": stage1_grp,
            "stage1_close": stage1_close, "stage2": stage2, "stage3": stage3,
            "cleanup_cm": _Cleanup()}


# ---------------- fusion ----------------
def _fusion(p, tap):
    nc, tc = p.nc, p.tc
    _graph_head(p, tap)
    with (
        tc.tile_pool(name="fu", bufs=2) as fu,
        tc.tile_pool(name="fup", bufs=2, space="PSUM") as fup,
    ):
        xcT = []
        for src_ in (p.g2, p.xt2):
            t = fu.tile([128, GPC], F32, tag=f"xcT{len(xcT)}", name=f"xcT{len(xcT)}")
            _dve_T(nc, t, src_[:], 128)
            xcT.append(t)
        c1 = fu.tile([GPC, 1024], F32, tag="c1", name="c1")
        w2 = fu.tile([128, 2, 1024], F32, tag="f1w", name="f1w")
        nc.sync.dma_start(out=w2[:], in_=p.fc1_w[:].rearrange(
            "(j p) n -> p j n", p=128))
        for n in range(2):
            ps = fup.tile([GPC, 512], F32, tag="mm", name="mm")
            for j in range(2):
                nc.tensor.matmul(ps[:], xcT[j][:],
                                 w2[:, j, n * 512:(n + 1) * 512],
                                 start=(j == 0), stop=(j == 1))
            nc.vector.tensor_copy(out=c1[:, n * 512:(n + 1) * 512], in_=ps[:])
        bb = fu.tile([GPC, 1024], F32, tag="fbb", name="fbb")
        nc.sync.dma_start(out=bb[:], in_=p.fc1_b[:])
        nc.vector.tensor_tensor(out=c1[:], in0=c1[:], in1=bb[:], op=OP.add)
        c1b = fu.tile([GPC, 1024], F32, tag="c1b", name="c1b")
        nc.scalar.activation(c1b[:], c1[:], ACT.Relu)
        c1T = [fu.tile([128, GPC], F32, tag=f"c1T{j}", name=f"c1T{j}") for j in range(8)]
        for j in range(8):
            _dve_T(nc, c1T[j], c1b[:, j * 128:(j + 1) * 128], 128)
        ps = fup.tile([GPC, 256], F32, tag="mm", name="mm")
        wf2 = fu.tile([128, 8, 256], F32, tag="f2w", name="f2w")
        nc.sync.dma_start(out=wf2[:], in_=p.fc2_w[:].rearrange(
            "(j p) n -> p j n", p=128))
        for j in range(8):
            nc.tensor.matmul(ps[:], c1T[j][:], wf2[:, j, :], start=(j == 0),
                             stop=(j == 7))
        c2 = fu.tile([GPC, 256], F32, tag="c2", name="c2")
        bb2 = fu.tile([GPC, 256], F32, tag="fbb2", name="fbb2")
        nc.sync.dma_start(out=bb2[:], in_=p.fc2_b[:])
        nc.vector.tensor_tensor(out=c2[:], in0=ps[:], in1=bb2[:], op=OP.add)
        c2b = fu.tile([GPC, 256], F32, tag="c2b", name="c2b")
        nc.scalar.activation(c2b[:], c2[:], ACT.Relu)
        c2T = []
        for j in range(2):
            t = fu.tile([128, GPC], F32, tag=f"c2T{j}", name=f"c2T{j}")
            _dve_T(nc, t, c2b[:, j * 128:(j + 1) * 128], 128)
            c2T.append(t)
        ow = fu.tile([128, 2], F32, tag="ow", name="ow")
        for j in range(2):
            nc.sync.dma_start(out=ow[:, j:j + 1], in_=p.out_w[j * 128:(j + 1) * 128, :])
        ps = fup.tile([GPC, 1], F32, tag="mm", name="mm")
        for j in range(2):
            nc.tensor.matmul(ps[:], c2T[j][:], ow[:, j:j + 1],
                             start=(j == 0), stop=(j == 1))
        o = fu.tile([GPC, 1], F32, tag="o", name="o")
        nc.vector.tensor_copy(out=o[:], in_=ps[:])
        nc.sync.dma_start(out=p.out[:], in_=o[:])


# ------------------------------------------------------------------ entry
def _build_and_run(inputs, taps=()):
    T_blocks, in_maps, out_b = _host_prep(inputs)
    nc, p = build_program(T_blocks, taps=taps)
    res = run_bass_kernel_spmd(nc, in_maps, list(range(NCORES)))
    return res, out_b, p


def kernel(**inputs) -> np.ndarray:
    res, out_b, _ = _build_and_run(inputs)
    out = np.concatenate([res.results[c]["out"] for c in range(NCORES)], axis=0)
    return (out + out_b).astype(np.float32)


# revision 46
# speedup vs baseline: 1.0146x; 1.0146x over previous
"""GATNet (3x GATConv graph branch + 1D-CNN protein branch + fusion MLP) on 8
Trainium2 NeuronCores via Bass/Tile.

Sharding: nodes row-sharded 1280/core (= 32 graphs/core since batch is sorted
blocks of 40); CNN branch sharded by the same 32 samples/core; weights
replicated in bf16.

Per GAT layer l:
  1. h = x @ [W | W@as_blk | W@ad_blk]  (m-outer node-block loop; attention
     scalars appear as extra columns). Augmented rows (h | a_s as f32
     bitcast | const 1) are written to local DRAM block by block.
  2. Chunked AllGather: every 2 node blocks fire their own AllGather into a
     block-major shared h_full, overlapping collectives with the remaining
     x@W compute. Gather indices are host-remapped to the chunked layout.
  3. Per 128-dst block: dst-sorted edge tiles; indirect-DMA gathers src rows
     into one batched tile; block-batched attention math (one add / Prelu /
     Exp chain over all tiles); numerator+denominator via one-hot S matmuls
     (S pre-scaled by exp for H<=2 layers, features pre-scaled for the
     10-head layer); scale by reciprocal; transpose tiles; bias+activation
     on transposed tiles -> next lhsT.

Self-contained: hardcodes all shapes; builds the per-call edge structure into
the traced program, compiles and runs via run_bass_kernel_spmd.
"""
import numpy as np
import ml_dtypes

import concourse.bass as bass
import concourse.mybir as mybir
import concourse.tile as tile
from concourse.bass_utils import run_bass_kernel_spmd
from concourse.masks import make_identity
from concourse.tile import add_dep_helper

NCORES = 8
N_NODES = 10240
N_GRAPHS = 256
NPC = N_NODES // NCORES          # 1280 nodes/core
GPC = N_GRAPHS // NCORES         # 32 graphs/core
NPG = N_NODES // N_GRAPHS        # 40 nodes/graph
BPC = NPC // 128                 # 10 dst blocks/core
# AllGather chunking per layer: lists of node-block counts per collective.
# L1's x@W is tiny, so one collective; deeper layers pipeline more chunks.
AG_CHUNKS = [[10], [4, 3, 3], [3, 3, 2, 2]]
SEQ = 1000
VOCAB = 26
EMB = 128
NEG_SLOPE = 0.2

F32 = mybir.dt.float32
F32R = mybir.dt.float32r
BF16 = mybir.dt.bfloat16
I32 = mybir.dt.int32
AX = mybir.AxisListType
OP = mybir.AluOpType
ACT = mybir.ActivationFunctionType

# (F_in, F_out, heads)
LAYERS = [(78, 780, 10), (780, 1560, 2), (1560, 3120, 1)]
# x@W psum chunk lists (cover F_out + 2H cols; head-aligned where needed)
MM_CHUNKS = [[390, 390, 20], [390, 390, 390, 390, 4], [448] * 6 + [434]]
# message-pass numerator chunks: per-head col ranges for S-scaled layers
MP_HEAD_CHUNKS = {1: [(0, 512), (512, 780)],
                  2: [(0, 512), (512, 1024), (1024, 1536), (1536, 2048),
                      (2048, 2560), (2560, 3072), (3072, 3120)]}

bf = lambda a: np.ascontiguousarray(a).astype(ml_dtypes.bfloat16)
f32 = lambda a: np.ascontiguousarray(a, dtype=np.float32)
cdiv = lambda a, b: -(-a // b)


# ------------------------------------------------------------------ walrus patch
def _split_sync_waits(nc, max_keep=1):
    for f in nc.m.functions:
        for bb in f.blocks:
            out, changed = [], False
            for ins in bb.instructions:
                si = ins.sync_info
                waits = list(si.on_wait) if si is not None and si.on_wait else []
                if len(waits) > max_keep:
                    extra, keep = waits[:-max_keep], waits[-max_keep:]
                    for i in range(0, len(extra), max_keep):
                        out.append(mybir.InstNoOp(
                            name=f"WSPLIT-{nc.next_id()}", engine=ins.engine,
                            bass_nofuse=True,
                            sync_info=mybir.SyncInfo(on_wait=extra[i:i + max_keep],
                                                     on_update=[])))
                    si.on_wait = keep
                    changed = True
                out.append(ins)
            if changed:
                bb.instructions[:] = out


# ------------------------------------------------------------------ host prep
def _remap_rows(n, li):
    """Global node id -> row in layer li's chunk-major h_full layout."""
    c, j = n // NPC, n % NPC
    m = j // 128
    b0s = np.cumsum([0] + AG_CHUNKS[li][:-1])
    q = np.searchsorted(b0s, m, side="right") - 1
    b0 = b0s[q]
    nb = np.asarray(AG_CHUNKS[li])[q]
    return b0 * (NCORES * 128) + c * (nb * 128) + (j - b0 * 128)


def _edge_structure(edge_index):
    src, dst = edge_index[0].astype(np.int64), edge_index[1].astype(np.int64)
    loop = np.arange(N_NODES, dtype=np.int64)
    s_all = np.concatenate([src, loop])
    d_all = np.concatenate([dst, loop])
    order = np.argsort(d_all, kind="stable")
    s_s, d_s = s_all[order], d_all[order]

    bounds = np.searchsorted(d_s, np.arange(0, N_NODES + 1, 128))
    cnt = bounds[1:] - bounds[:-1]
    tiles_needed = -(-cnt // 128)
    T_blocks = [int(tiles_needed.reshape(NCORES, BPC)[:, p].max()) for p in range(BPC)]
    t_off = np.cumsum([0] + T_blocks)
    T_tot = int(t_off[-1])

    src_idx = np.zeros((NCORES, 3, T_tot, 128), np.int32)
    S = np.zeros((NCORES, T_tot, 128, 128), np.float32)
    for c in range(NCORES):
        for p_ in range(BPC):
            blk = c * BPC + p_
            e0, e1 = int(bounds[blk]), int(bounds[blk + 1])
            m = e1 - e0
            ti = np.arange(m) // 128 + t_off[p_]
            ei = np.arange(m) % 128
            for li in range(3):
                src_idx[c, li, ti, ei] = _remap_rows(s_s[e0:e1], li)
            S[c, ti, ei, d_s[e0:e1] - 128 * blk] = 1.0
    ST = np.swapaxes(S, 2, 3)
    SS = np.concatenate([S, ST], axis=3)          # [8, T, 128, 256]
    # [8, 128, 3*T_tot]
    src_idxT = np.ascontiguousarray(
        np.swapaxes(src_idx.reshape(NCORES, 3 * T_tot, 128), 1, 2))
    return T_blocks, src_idxT, bf(SS)


def _aug_w(W, a_s, a_d, H):
    """[W | W@as_blk | W@ad_blk] with as_blk[f,h] = a_s[h, f - h*FH]."""
    fi, fo = W.shape
    FH = fo // H
    was = np.zeros((fi, H), np.float32)
    wad = np.zeros((fi, H), np.float32)
    for h in range(H):
        was[:, h] = W[:, h * FH:(h + 1) * FH] @ a_s[h]
        wad[:, h] = W[:, h * FH:(h + 1) * FH] @ a_d[h]
    return np.concatenate([W, was, wad], axis=1)


def _bias_colmajor(b, fo):
    n_t = cdiv(fo, 128)
    pad = np.zeros(n_t * 128, np.float32)
    pad[:fo] = b
    return np.ascontiguousarray(pad.reshape(n_t, 128).T)   # [128, n_t]


def _host_prep(inputs):
    ii = {k: np.asarray(v) for k, v in inputs.items()}
    T_blocks, src_idxT, SS = _edge_structure(ii["edge_index"])

    xT = np.ascontiguousarray(np.swapaxes(f32(ii["x"]), 0, 1))   # [78, 10240]

    W_aug, b_col = [], []
    for i, (fi, fo, H) in enumerate(LAYERS):
        W_aug.append(bf(_aug_w(f32(ii[f"W{i+1}"]), f32(ii[f"as{i+1}"]),
                               f32(ii[f"ad{i+1}"]), H)))
        b_col.append(_bias_colmajor(f32(ii[f"b{i+1}"]).reshape(-1), fo))

    cw1 = f32(ii["cw1"])
    cw1f = np.zeros((125, 8, 2, 128), np.float32)
    for sc in range(8):
        for ks in range(2):
            blk = cw1[:, sc * 125:(sc + 1) * 125, ks * 4:(ks + 1) * 4]
            cw1f[:, sc, ks, :] = blk.transpose(1, 2, 0).reshape(125, 128)
    cwT = lambda w: np.ascontiguousarray(np.transpose(f32(ii[w]), (1, 2, 0)))

    w1xt = np.ascontiguousarray(
        f32(ii["fc1_xt_w"]).reshape(128, 33, 1024).transpose(1, 0, 2))

    emb_f = f32(ii["emb_xt"])                                   # [26, 128]
    rep = lambda a, n: np.ascontiguousarray(
        np.broadcast_to(f32(a).reshape(1, -1), (n, f32(a).size)))

    shared = {
        "W1": W_aug[0], "W2": W_aug[1], "W3": W_aug[2],
        "bc1": b_col[0], "bc2": b_col[1], "bc3": b_col[2],
        "fc_g1_w": bf(ii["fc_g1_w"]), "fc_g1_b": rep(ii["fc_g1_b"], GPC),
        "fc_g2_w": f32(ii["fc_g2_w"]), "fc_g2_b": rep(ii["fc_g2_b"], GPC),
        "cw1f": bf(cw1f), "cb1": f32(ii["cb1"]).reshape(-1, 1),
        "cw2T": bf(cwT("cw2")), "cb2": f32(ii["cb2"]).reshape(-1, 1),
        "cw3T": bf(cwT("cw3")), "cb3": f32(ii["cb3"]).reshape(-1, 1),
        "cw4T": bf(cwT("cw4")), "cb4": f32(ii["cb4"]).reshape(-1, 1),
        "w1xt": bf(w1xt), "fc1_xt_b": rep(ii["fc1_xt_b"], GPC),
        "fc2_xt_w": f32(ii["fc2_xt_w"]), "fc2_xt_b": rep(ii["fc2_xt_b"], GPC),
        "fc1_w": f32(ii["fc1_w"]), "fc1_b": rep(ii["fc1_b"], GPC),
        "fc2_w": f32(ii["fc2_w"]), "fc2_b": rep(ii["fc2_b"], GPC),
        "out_w": f32(ii["out_w"]),
    }
    in_maps = []
    for c in range(NCORES):
        m = dict(shared)
        m["xT"] = bf(xT[:, c * NPC:(c + 1) * NPC])
        m["esrcT"] = src_idxT[c]
        m["SS"] = SS[c]
        tgt_c = ii["target"][c * GPC:(c + 1) * GPC]               # [32, 1000]
        E_full = emb_f[tgt_c]                                     # [32, 1000, 128]
        E2 = E_full.reshape(8, 4, 8, 125, 128).transpose(2, 0, 3, 1, 4)
        m["E2"] = bf(E2.reshape(8, 8, 125, 512))
        in_maps.append(m)
    out_b = float(np.asarray(ii["out_b"]).reshape(-1)[0])
    return T_blocks, in_maps, out_b


# ------------------------------------------------------------------ program
class P:
    pass


def _aug_cols(li):
    fo, H = LAYERS[li][1], LAYERS[li][2]
    return fo + 2 * H + 2        # h | a_s(f32 as 2H bf16) | ones | pad


def build_program(T_blocks, taps=()):
    T_tot = sum(T_blocks)
    nc = bass.Bass()
    p = P()
    p.nc = nc
    p.taps = set(taps)
    p.tap_tensors = {}

    dp = lambda name, shape, dt: nc.declare_dram_parameter(name, list(shape), dt,
                                                           isOutput=False)
    p.xT = dp("xT", [78, NPC], BF16)
    p.W = [dp(f"W{i+1}", [LAYERS[i][0], LAYERS[i][1] + 2 * LAYERS[i][2]], BF16)
           for i in range(3)]
    p.bc = [dp(f"bc{i+1}", [128, cdiv(LAYERS[i][1], 128)], F32) for i in range(3)]
    p.esrcT = dp("esrcT", [128, 3 * T_tot], I32)
    p.SS = dp("SS", [T_tot, 128, 256], BF16)
    p.fc_g1_w = dp("fc_g1_w", [3120, 1024], BF16)
    p.fc_g1_b = dp("fc_g1_b", [GPC, 1024], F32)
    p.fc_g2_w = dp("fc_g2_w", [1024, 128], F32)
    p.fc_g2_b = dp("fc_g2_b", [GPC, 128], F32)
    p.E2 = dp("E2", [8, 8, 125, 512], BF16)
    p.cw1f = dp("cw1f", [125, 8, 2, 128], BF16)
    p.cb1 = dp("cb1", [32, 1], F32)
    p.cw2T = dp("cw2T", [32, 8, 64], BF16)
    p.cb2 = dp("cb2", [64, 1], F32)
    p.cw3T = dp("cw3T", [64, 8, 96], BF16)
    p.cb3 = dp("cb3", [96, 1], F32)
    p.cw4T = dp("cw4T", [96, 8, 128], BF16)
    p.cb4 = dp("cb4", [128, 1], F32)
    p.w1xt = dp("w1xt", [33, 128, 1024], BF16)
    p.fc1_xt_b = dp("fc1_xt_b", [GPC, 1024], F32)
    p.fc2_xt_w = dp("fc2_xt_w", [1024, 128], F32)
    p.fc2_xt_b = dp("fc2_xt_b", [GPC, 128], F32)
    p.fc1_w = dp("fc1_w", [256, 1024], F32)
    p.fc1_b = dp("fc1_b", [GPC, 1024], F32)
    p.fc2_w = dp("fc2_w", [1024, 256], F32)
    p.fc2_b = dp("fc2_b", [GPC, 256], F32)
    p.out_w = dp("out_w", [256, 1], F32)
    p.out = nc.declare_dram_parameter("out", [GPC, 1], F32, isOutput=True)

    p.h_loc = [nc.dram_tensor(f"h{i+1}_loc", [NPC, _aug_cols(i)], BF16)
               for i in range(3)]
    p.h_full = [nc.dram_tensor(f"h{i+1}_full", [N_NODES, _aug_cols(i)], BF16,
                               addr_space="Shared") for i in range(3)]

    def tap(name, shape, dt=F32):
        if name in p.taps:
            t = nc.declare_dram_parameter("tap_" + name, list(shape), dt,
                                          isOutput=True)
            p.tap_tensors[name] = t
            return t
        return None

    with tile.TileContext(nc) as tc:
        p.tc = tc
        _cp_cm = tc.tile_pool(name="const", bufs=1)
        const_pool = _cp_cm.__enter__()
        p.ident = const_pool.tile([128, 128], BF16)
        make_identity(nc, p.ident[:])
        p.ones_col = const_pool.tile([128, 1], BF16, tag="ones1", name="ones1")
        nc.vector.memset(p.ones_col[:], 1.0)
        p.join_dummy = const_pool.tile([1, 1], F32, tag="jd", name="jd")
        p.head_pool = const_pool

        stages = _cnn_make(p, tap)
        p.cnn_stages = stages
        _gat_branch(p, T_blocks, tap)
        _fusion(p, tap)
        for cm in p.gat_cleanup:
            cm.__exit__(None, None, None)
        _cp_cm.__exit__(None, None, None)

    _split_sync_waits(nc)
    return nc, p


# ---------------- GAT branch ----------------
def _gat_branch(p, T_blocks, tap):
    nc, tc = p.nc, p.tc

    mpc_cm = tc.tile_pool(name="mpc", bufs=1)
    mpc_pool = mpc_cm.__enter__()
    eidx = mpc_pool.tile([128, 3 * sum(T_blocks)], I32, tag="eidx", name="eidx")
    nc.sync.dma_start(out=eidx[:], in_=p.esrcT[:])
    p.eidx = eidx
    adp_cms = [tc.tile_pool(name=f"adp{li}", bufs=1) for li in range(3)]
    adp_pools = [cm.__enter__() for cm in adp_cms]

    xT_cm = tc.tile_pool(name="xT0", bufs=1)
    xT_pool = xT_cm.__enter__()
    xT_tiles = [xT_pool.tile([78, NPC], BF16, tag="x0", name="x0")]
    nc.sync.dma_start(out=xT_tiles[0][:], in_=p.xT[:])

    for li, (fi, fo, H) in enumerate(LAYERS):
        is_last = li == 2
        n_k = cdiv(fi, 128)
        cols = _aug_cols(li)
        a_d_pool = adp_pools[li]
        a_d_tiles = []
        chunks = MM_CHUNKS[li]
        offs = [int(v) for v in np.cumsum([0] + chunks)]
        n_ch = len(chunks)
        ag_insts = []
        with (
            tc.tile_pool(name=f"w{li}", bufs=1) as wpool,
            tc.tile_pool(name=f"mm{li}", bufs=2) as mpool,
            tc.tile_pool(name=f"mmp{li}", bufs=2 if li == 0 else 1,
                         space="PSUM") as pspool,
        ):
            W_sb = []
            for k in range(n_k):
                kp = min(128, fi - k * 128)
                t = wpool.tile([kp, offs[-1]], BF16, tag=f"W{k}", name=f"W{k}")
                nc.sync.dma_start(out=t[:], in_=p.W[li][k * 128:k * 128 + kp, :])
                W_sb.append(t)
            pending_writes = []
            for m in range(BPC):
                psums = [pspool.tile([128, chunks[n]], F32, tag=f"hp{n}",
                                     name=f"hp{n}")
                         for n in range(n_ch)]
                for k in range(n_k):
                    kp = min(128, fi - k * 128)
                    lhs = xT_tiles[k][:kp, m * 128:(m + 1) * 128]
                    for n in range(n_ch):
                        nc.tensor.matmul(
                            psums[n][:], lhs, W_sb[k][:, offs[n]:offs[n + 1]],
                            start=(k == 0), stop=(k == n_k - 1))
                stage = mpool.tile([128, cols], BF16, tag="stage", name="stage")
                for n in range(n_ch):
                    lo, hi = offs[n], offs[n + 1]
                    if hi <= fo:
                        nc.scalar.copy(out=stage[:, lo:hi], in_=psums[n][:])
                    else:
                        if lo < fo:
                            nc.scalar.copy(out=stage[:, lo:fo],
                                           in_=psums[n][:, :fo - lo])
                        a_sf = mpool.tile([128, H], F32, tag="a_sf", name="a_sf")
                        nc.vector.tensor_copy(
                            out=a_sf[:], in_=psums[n][:, fo - lo:fo - lo + H])
                        a_d = a_d_pool.tile([128, H], BF16, tag=f"a_d{m}",
                                            name=f"a_d{m}")
                        nc.vector.tensor_copy(
                            out=a_d[:],
                            in_=psums[n][:, fo - lo + H:fo - lo + 2 * H])
                        a_d_tiles.append(a_d)
                        nc.vector.tensor_copy(
                            out=stage[:, fo:fo + 2 * H],
                            in_=a_sf[:].bitcast(BF16))
                oc = fo + 2 * H
                nc.vector.memset(stage[:, oc:oc + 1], 1.0)
                nc.vector.memset(stage[:, oc + 1:cols], 0.0)
                w = nc.scalar.dma_start(
                    out=p.h_loc[li][m * 128:(m + 1) * 128, :], in_=stage[:])
                pending_writes.append(w)
                b0s = [int(v) for v in np.cumsum([0] + AG_CHUNKS[li][:-1])]
                if m + 1 - b0s[len(ag_insts)] == AG_CHUNKS[li][len(ag_insts)]:
                    b0, nb = b0s[len(ag_insts)], AG_CHUNKS[li][len(ag_insts)]
                    cc = nc.gpsimd.collective_compute(
                        "AllGather", OP.bypass,
                        replica_groups=[list(range(NCORES))],
                        ins=[p.h_loc[li][b0 * 128:(b0 + nb) * 128, :]],
                        outs=[p.h_full[li][b0 * NCORES * 128:
                                           (b0 + nb) * NCORES * 128, :]])
                    for w_ in pending_writes:
                        add_dep_helper(cc.ins, w_.ins, reason="AG waits h_loc")
                    pending_writes = []
                    ag_insts.append(cc)

        # join: one gpsimd op that waits for all AG chunks of this layer
        join = nc.gpsimd.memset(p.join_dummy[:], float(li))
        for cc in ag_insts:
            add_dep_helper(join.ins, cc.ins, reason="join waits AG chunk")

        t = tap(f"h{li+1}", [NPC, cols], BF16)
        if t is not None:
            d = nc.sync.dma_start(out=t[:], in_=p.h_loc[li][:])
            add_dep_helper(d.ins, join.ins, reason="tap waits AGs")

        xT_cm.__exit__(None, None, None)

        filler = None
        if li == 0:
            p.cnn_stages["stage1_open"]()

            def filler(blk):
                if blk < 8:
                    p.cnn_stages["stage1_grp"](blk)
                elif blk == 8:
                    p.cnn_stages["stage1_close"]()
        elif li == 1:
            p.cnn_stages["stage2"]()
        elif li == 2:
            p.cnn_stages["stage3"]()

        n_kT = cdiv(fo, 128)
        xTn_cm = tc.tile_pool(name=f"xTn{li}", bufs=1)
        xTn_pool = xTn_cm.__enter__()
        xT_out = []
        for j in range(n_kT):
            kp = min(128, fo - j * 128)
            xT_out.append(xTn_pool.tile([kp, NPC], BF16, tag=f"xT{li}_{j}",
                                        name=f"xT{li}_{j}"))

        _message_pass(p, T_blocks, li, a_d_tiles, join, xT_out, filler)

        t = tap(f"xT{li+2}" if not is_last else "o3T", [fo, NPC], BF16)
        if t is not None:
            for j in range(n_kT):
                kp = min(128, fo - j * 128)
                nc.sync.dma_start(out=t[j * 128:j * 128 + kp, :], in_=xT_out[j][:])

        xT_tiles = xT_out
        xT_cm = xTn_cm
        if is_last:
            p.out3T = xT_out
            p.gat_cleanup = [xTn_cm, p.cnn_stages["cleanup_cm"],
                             *reversed(adp_cms), mpc_cm]
    return


def _message_pass(p, T_blocks, li, a_d_tiles, ag_join, xT_out, filler=None):
    nc, tc = p.nc, p.tc
    fi, fo, H = LAYERS[li]
    FH = fo // H
    cols = _aug_cols(li)
    t_off = np.cumsum([0] + T_blocks)
    n_kT = cdiv(fo, 128)
    s_scale = li > 0                 # scale S by exp for H<=2; scale g for H=10
    nsub = 1 if li == 0 else 2

    with (
        tc.tile_pool(name=f"mp{li}", bufs=2) as mp,
        tc.tile_pool(name=f"mpS{li}", bufs=2) as mpS,
        tc.tile_pool(name=f"bc{li}", bufs=1) as bcp,
    ):
        bcol = bcp.tile([128, n_kT], F32, tag="bcol", name="bcol")
        nc.sync.dma_start(out=bcol[:], in_=p.bc[li][:])

        for blk in range(BPC):
            if filler is not None:
                filler(blk)
            Tb = T_blocks[blk]
            t0 = int(t_off[blk])
            e0 = li * int(t_off[-1]) + t0
            subs = [(0, Tb)] if nsub == 1 else \
                [(0, (Tb + 1) // 2), ((Tb + 1) // 2, Tb - (Tb + 1) // 2)]
            rows_bf = mp.tile([128, fo], BF16, tag="rows_bf", name="rows_bf")
            rec = mp.tile([128, H], F32, tag="rec", name="rec")
            with (
                tc.tile_pool(name=f"op{li}_{blk}", bufs=1, space="PSUM") as pp,
            ):
                if s_scale:
                    head_chunks = MP_HEAD_CHUNKS[li]
                    n_hc = len(head_chunks)
                    dlast = head_chunks[-1][1] - head_chunks[-1][0]
                    opsum = {}
                    for h in range(H):
                        for ci, (lo, hi) in enumerate(head_chunks):
                            w_ = hi - lo + (1 if ci == n_hc - 1 else 0)
                            opsum[(h, ci)] = pp.tile(
                                [128, w_], F32,
                                tag=f"op{h}_{ci}", name=f"op{h}_{ci}")
                else:
                    num0 = pp.tile([128, 390], F32, tag="num0", name="num0")
                    num1 = pp.tile([128, 390], F32, tag="num1", name="num1")
                    dn = pp.tile([128, H], F32, tag="dn", name="dn")
                aux = pp.tile([128, Tb * H], F32, tag="aux", name="aux")

                for si, (ta, tn) in enumerate(subs):
                    is_fs, is_ls = si == 0, si == nsub - 1
                    SS_sub = mpS.tile([128, tn, 256], BF16, tag=f"SS{si}",
                                      name=f"SS{si}")
                    nc.sync.dma_start(
                        out=SS_sub[:],
                        in_=p.SS[t0 + ta:t0 + ta + tn].rearrange(
                            "t p c -> p t c"))
                    g_sub = mp.tile([128, tn, cols], BF16, tag=f"g{si}",
                                    name=f"g{si}", bufs=3)
                    for t in range(tn):
                        gi = nc.gpsimd.indirect_dma_start(
                            out=g_sub[:, t, :], out_offset=None,
                            in_=p.h_full[li][:],
                            in_offset=bass.IndirectOffsetOnAxis(
                                ap=p.eidx[:, e0 + ta + t:e0 + ta + t + 1],
                                axis=0))
                        add_dep_helper(gi.ins, ag_join.ins,
                                       reason="gather waits AG")
                    for t in range(tn):
                        nc.tensor.matmul(
                            aux[:, (ta + t) * H:(ta + t + 1) * H],
                            SS_sub[:, t, 128:256],
                            a_d_tiles[blk][:], start=True, stop=True)
                    sc = mp.tile([128, tn * H], F32, tag=f"sc{si}",
                                 name=f"sc{si}")
                    nc.vector.tensor_tensor(
                        out=sc[:].rearrange("p (t h) -> p t h", h=H),
                        in0=g_sub[:, :, fo:fo + 2 * H].bitcast(F32),
                        in1=aux[:, ta * H:(ta + tn) * H].rearrange(
                            "p (t h) -> p t h", h=H),
                        op=OP.add)
                    nc.scalar.activation(sc[:], sc[:], ACT.Prelu,
                                         alpha=NEG_SLOPE)
                    ex = mp.tile([128, tn * H], BF16, tag=f"ex{si}",
                                 name=f"ex{si}")
                    nc.scalar.activation(ex[:], sc[:], ACT.Exp)
                    ex3 = ex[:].rearrange("p (t h) -> p t h", h=H)

                    if s_scale:
                        Ssc = []
                        for h in range(H):
                            sh = mp.tile([128, tn * 128], BF16,
                                         tag=f"Ssc{si}_{h}", name=f"Ssc{si}_{h}")
                            nc.vector.tensor_tensor(
                                out=sh[:].rearrange("p (t c) -> p t c", c=128),
                                in0=SS_sub[:, :, 0:128],
                                in1=ex3[:, :, h:h + 1].broadcast_to(
                                    [128, tn, 128]),
                                op=OP.mult)
                            Ssc.append(sh)
                        # denominators first (own their bank's initial clear)
                        for h in range(H):
                            for t in range(tn):
                                nc.tensor.matmul(
                                    opsum[(h, n_hc - 1)][:, dlast:dlast + 1],
                                    Ssc[h][:, t * 128:(t + 1) * 128],
                                    p.ones_col[:],
                                    start=(is_fs and t == 0),
                                    stop=(is_ls and t == tn - 1),
                                    skip_group_check=not is_fs)
                        if is_ls:
                            for h in range(H):
                                nc.vector.tensor_scalar(
                                    out=rec[:, h:h + 1],
                                    in0=opsum[(h, n_hc - 1)][:,
                                              dlast:dlast + 1],
                                    scalar1=1e-16, scalar2=None, op0=OP.add)
                            nc.vector.reciprocal(rec[:], rec[:])
                        # numerators chunk-major within this sub-unit; the
                        # last chunk's bank was cleared by the denominator
                        # group -> start=False overwrite-on-cleared
                        for ci, (lo, hi) in enumerate(head_chunks):
                            is_dl = ci == n_hc - 1
                            for h in range(H):
                                for t in range(tn):
                                    nc.tensor.matmul(
                                        opsum[(h, ci)][:, :hi - lo],
                                        Ssc[h][:, t * 128:(t + 1) * 128],
                                        g_sub[:, t, h * FH + lo:h * FH + hi],
                                        start=(is_fs and t == 0 and not is_dl),
                                        stop=(is_ls and t == tn - 1),
                                        skip_group_check=is_dl or not is_fs)
                                if is_ls:
                                    nc.vector.tensor_scalar(
                                        out=rows_bf[:,
                                                    h * FH + lo:h * FH + hi],
                                        in0=opsum[(h, ci)][:, :hi - lo],
                                        scalar1=rec[:, h:h + 1], scalar2=None,
                                        op0=OP.mult)
                    else:
                        # one 4D broadcast multiply scales all heads at once
                        gs = mp.tile([128, tn, fo + H], BF16, tag=f"gs{si}",
                                     name=f"gs{si}", bufs=3)
                        nc.vector.tensor_tensor(
                            out=gs[:, :, 0:fo].rearrange(
                                "p t (h f) -> p t h f", f=FH),
                            in0=g_sub[:, :, 0:fo].rearrange(
                                "p t (h f) -> p t h f", f=FH),
                            in1=ex3.unsqueeze(3).broadcast_to(
                                [128, tn, H, FH]),
                            op=OP.mult)
                        nc.vector.tensor_copy(out=gs[:, :, fo:fo + H], in_=ex3)
                        for t in range(tn):
                            nc.tensor.matmul(
                                dn[:], SS_sub[:, t, 0:128],
                                gs[:, t, fo:fo + H],
                                start=(is_fs and t == 0),
                                stop=(is_ls and t == tn - 1))
                        if is_ls:
                            nc.vector.tensor_scalar(
                                out=rec[:], in0=dn[:], scalar1=1e-16,
                                scalar2=None, op0=OP.add)
                            nc.vector.reciprocal(rec[:], rec[:])
                        for ni, (nt, lo, hi) in enumerate(
                                [(None, 0, 390), (None, 390, 780)]):
                            tgt = num0 if ni == 0 else num1
                            for t in range(tn):
                                nc.tensor.matmul(
                                    tgt[:], SS_sub[:, t, 0:128],
                                    gs[:, t, lo:hi],
                                    start=(is_fs and t == 0),
                                    stop=(is_ls and t == tn - 1))
                            if is_ls:
                                nh = 5
                                h0 = 0 if ni == 0 else 5
                                nc.vector.tensor_tensor(
                                    out=rows_bf[:, lo:hi].rearrange(
                                        "p (h f) -> p h f", f=FH),
                                    in0=tgt[:].rearrange(
                                        "p (h f) -> p h f", f=FH),
                                    in1=rec[:, h0:h0 + nh].unsqueeze(
                                        2).broadcast_to([128, nh, FH]),
                                    op=OP.mult)
            if s_scale:
                with tc.tile_pool(name=f"tp{li}_{blk}", bufs=2,
                                  space="PSUM") as ptp:
                    for j in range(n_kT):
                        kp = min(128, fo - j * 128)
                        tp = ptp.tile([kp, 128], BF16, tag="tp", name="tp")
                        nc.tensor.transpose(
                            tp[:], rows_bf[:, j * 128:j * 128 + kp], p.ident[:])
                        nc.scalar.activation(
                            xT_out[j][:, blk * 128:(blk + 1) * 128], tp[:],
                            ACT.Relu, bias=bcol[:kp, j:j + 1])
            else:
                # transpose + batched ELU epilogue
                with tc.tile_pool(name=f"tp{li}_{blk}", bufs=2,
                                  space="PSUM") as ptp:
                    zall = mp.tile([128, n_kT * 128], F32, tag="zall",
                                   name="zall")
                    for j in range(n_kT):
                        kp = min(128, fo - j * 128)
                        tp = ptp.tile([kp, 128], BF16, tag="tp", name="tp")
                        nc.tensor.transpose(
                            tp[:], rows_bf[:, j * 128:j * 128 + kp], p.ident[:])
                        nc.scalar.activation(
                            zall[:kp, j * 128:(j + 1) * 128], tp[:],
                            ACT.Identity, bias=bcol[:kp, j:j + 1])
                    t1 = mp.tile([128, n_kT * 128], F32, tag="elu1", name="elu1")
                    nc.vector.tensor_scalar(out=t1[:], in0=zall[:], scalar1=0.0,
                                            scalar2=None, op0=OP.min)
                    nc.scalar.activation(t1[:], t1[:], ACT.Exp)
                    nc.scalar.activation(zall[:], zall[:], ACT.Relu)
                    for j in range(n_kT):
                        kp = min(128, fo - j * 128)
                        nc.vector.scalar_tensor_tensor(
                            out=xT_out[j][:, blk * 128:(blk + 1) * 128],
                            in0=zall[:kp, j * 128:(j + 1) * 128], scalar=-1.0,
                            in1=t1[:kp, j * 128:(j + 1) * 128],
                            op0=OP.add, op1=OP.add)


def _dve_T(nc, dst, src, n):
    """dst[n, 32] = src[32, n].T via DVE 32x32 block transposes."""
    for i in range(n // 32):
        nc.vector.transpose(out=dst[32 * i:32 * (i + 1), :],
                            in_=src[:, 32 * i:32 * (i + 1)])


# ---------------- graph head ----------------
def _graph_head(p, tap):
    nc, tc = p.nc, p.tc
    n_kT = len(p.out3T)
    with (
        tc.tile_pool(name="gh", bufs=2) as gh,
        tc.tile_pool(name="ghG", bufs=1) as ghG,
        tc.tile_pool(name="ghp", bufs=2, space="PSUM") as ghp,
    ):
        gT = [ghG.tile([min(128, 3120 - j * 128), GPC], BF16, tag=f"gT{j}", name=f"gT{j}")
              for j in range(n_kT)]
        for j in range(n_kT):
            kp = min(128, 3120 - j * 128)
            gm = gh.tile([kp, GPC * 20], BF16, tag="gmx", name="gmx")
            v = p.out3T[j][:].rearrange("p (g n) -> p g n", n=NPG)
            nc.vector.tensor_tensor(
                out=gm[:].rearrange("p (g n) -> p g n", n=20),
                in0=v[:, :, 0:20], in1=v[:, :, 20:40], op=OP.max)
            nc.vector.reduce_max(
                gT[j][:], gm[:].rearrange("p (g n) -> p g n", n=20),
                axis=AX.X)
        g1 = ghG.tile([GPC, 1024], F32, tag="g1", name="g1")
        psn = [ghp.tile([GPC, 512], F32, tag=f"mm{n}", name=f"mm{n}", bufs=1)
               for n in range(2)]
        for j in range(n_kT):
            kp = min(128, 3120 - j * 128)
            w = gh.tile([kp, 1024], BF16, tag="fg1w", name="fg1w", bufs=3)
            nc.sync.dma_start(out=w[:], in_=p.fc_g1_w[j * 128:j * 128 + kp, :])
            for n in range(2):
                nc.tensor.matmul(psn[n][:], gT[j][:],
                                 w[:, n * 512:(n + 1) * 512], start=(j == 0),
                                 stop=(j == n_kT - 1))
        for n in range(2):
            nc.vector.tensor_copy(out=g1[:, n * 512:(n + 1) * 512],
                                  in_=psn[n][:])
        bb1 = gh.tile([GPC, 1024], F32, tag="ghbb", name="ghbb")
        nc.sync.dma_start(out=bb1[:], in_=p.fc_g1_b[:])
        nc.vector.tensor_tensor(out=g1[:], in0=g1[:], in1=bb1[:], op=OP.add)
        g1b = ghG.tile([GPC, 1024], F32, tag="g1b", name="g1b")
        nc.scalar.activation(g1b[:], g1[:], ACT.Relu)
        g1T = [ghG.tile([128, GPC], F32, tag=f"g1T{j}", name=f"g1T{j}") for j in range(8)]
        for j in range(8):
            _dve_T(nc, g1T[j], g1b[:, j * 128:(j + 1) * 128], 128)
        ps = ghp.tile([GPC, 128], F32, tag="mm", name="mm")
        w8 = gh.tile([128, 8, 128], F32, tag="fg2w", name="fg2w")
        nc.sync.dma_start(out=w8[:], in_=p.fc_g2_w[:].rearrange(
            "(j p) n -> p j n", p=128))
        for j in range(8):
            nc.tensor.matmul(ps[:], g1T[j][:], w8[:, j, :], start=(j == 0),
                             stop=(j == 7))
        p.g2 = p.head_pool.tile([GPC, 128], F32, tag="g2", name="g2")
        bb2 = gh.tile([GPC, 128], F32, tag="ghbb2", name="ghbb2")
        nc.sync.dma_start(out=bb2[:], in_=p.fc_g2_b[:])
        nc.vector.tensor_tensor(out=p.g2[:], in0=ps[:], in1=bb2[:], op=OP.add)
        t = tap("g2", [GPC, 128])
        if t is not None:
            nc.sync.dma_start(out=t[:], in_=p.g2[:])


# ---------------- CNN branch ----------------
def _cnn_make(p, tap):
    """CNN branch split into stages so the orchestrator can interleave them
    into the AllGather gaps. Pools open at stage1, closed via cleanup_cm."""
    nc, tc = p.nc, p.tc
    st = {}

    class _Cleanup:
        def __exit__(self, *a):
            for cm in st["cms"]:
                cm.__exit__(None, None, None)

    def stage1_open():
        cn_cm = tc.tile_pool(name="cn", bufs=3)
        cnw_cm = tc.tile_pool(name="cnw", bufs=1)
        cny_cm = tc.tile_pool(name="cny", bufs=1)
        cn = cn_cm.__enter__()
        cnw = cnw_cm.__enter__()
        cny = cny_cm.__enter__()
        st["cms"] = [cny_cm, cnw_cm, cn_cm]
        st["cn"], st["cnw"], st["cny"] = cn, cnw, cny

        cw1f_sb = cny.tile([125, 8, 2, 128], BF16, tag="cw1f", name="cw1f")
        nc.sync.dma_start(out=cw1f_sb[:], in_=p.cw1f[:])
        cw2_sb = cnw.tile([32, 8, 64], BF16, tag="cw2", name="cw2")
        nc.sync.dma_start(out=cw2_sb[:], in_=p.cw2T[:])
        cw3_sb = cnw.tile([64, 8, 96], BF16, tag="cw3", name="cw3")
        nc.sync.dma_start(out=cw3_sb[:], in_=p.cw3T[:])
        cw4_sb = cnw.tile([96, 8, 128], BF16, tag="cw4", name="cw4")
        nc.sync.dma_start(out=cw4_sb[:], in_=p.cw4T[:])
        cb = {}
        for nm, sh in [("cb1", 32), ("cb2", 64), ("cb3", 96), ("cb4", 128)]:
            cb[nm] = cnw.tile([sh, 1], F32, tag=nm, name=nm)
            nc.sync.dma_start(out=cb[nm][:], in_=getattr(p, nm)[:])
        st.update(cw1f=cw1f_sb, cw2=cw2_sb, cw3=cw3_sb, cw4=cw4_sb, cb=cb)
        st["y1"] = cny.tile([32, GPC * 121], BF16, tag="y1", name="y1")
        st["cnp1_cm"] = tc.tile_pool(name="cnp1", bufs=2, space="PSUM")
        st["cnp1"] = st["cnp1_cm"].__enter__()

    def stage1_grp(grp):
        cn, cnp, cb = st["cn"], st["cnp1"], st["cb"]
        cw1f_sb, y1 = st["cw1f"], st["y1"]
        pc = [cnp.tile([128, 512], F32, tag=f"pc{k}", name=f"pc{k}", bufs=1)
              for k in range(2)]
        for sc in range(8):
            E = cn.tile([125, 512], BF16, tag="E", name="E")
            nc.sync.dma_start(out=E[:], in_=p.E2[sc, grp])
            for ks in range(2):
                nc.tensor.matmul(pc[ks][:], cw1f_sb[:, sc, ks, :], E[:],
                                 start=(sc == 0), stop=(sc == 7))
        acc = cn.tile([32, 4 * 121], F32, tag="c1acc", name="c1acc")
        accr = acc[:].rearrange("p (b t) -> p b t", b=4)
        firstop = True
        for ks in range(2):
            for kl in range(4):
                k = ks * 4 + kl
                src = pc[ks][:].rearrange("p (b j) -> p b j", b=4)[
                    kl * 32:(kl + 1) * 32, :, k:k + 121]
                if firstop:
                    nc.vector.tensor_copy(out=accr, in_=src)
                    firstop = False
                else:
                    nc.vector.tensor_tensor(out=accr, in0=accr, in1=src,
                                            op=OP.add)
        nc.scalar.activation(y1[:, grp * 4 * 121:(grp + 1) * 4 * 121],
                             acc[:], ACT.Relu, bias=cb["cb1"][:32, :1])

    def stage1_close():
        st["cnp1_cm"].__exit__(None, None, None)

    def stage2():
        cn, cny, cb = st["cn"], st["cny"], st["cb"]
        cw2_sb, cw3_sb, cw4_sb = st["cw2"], st["cw3"], st["cw4"]
        y1 = st["y1"]
        with tc.tile_pool(name="cnp2", bufs=2, space="PSUM") as cnp:
            y2 = cny.tile([64, GPC * 114], BF16, tag="y2", name="y2")
            for grp in range(8):
                ps = cnp.tile([64, 4 * 114], F32, tag="pc0", name="pc0")
                for k in range(8):
                    rhs = y1[:].rearrange("p (b t) -> p b t", t=121)[
                        :, grp * 4:(grp + 1) * 4, k:k + 114]
                    nc.tensor.matmul(ps[:], cw2_sb[:, k, :], rhs, start=(k == 0),
                                     stop=(k == 7))
                nc.scalar.activation(y2[:, grp * 4 * 114:(grp + 1) * 4 * 114], ps[:],
                                     ACT.Relu, bias=cb["cb2"][:, :1])
            y3 = cny.tile([96, GPC * 107], BF16, tag="y3", name="y3")
            for grp in range(8):
                ps = cnp.tile([96, 4 * 107], F32, tag="pc0", name="pc0")
                for k in range(8):
                    rhs = y2[:].rearrange("p (b t) -> p b t", t=114)[
                        :, grp * 4:(grp + 1) * 4, k:k + 107]
                    nc.tensor.matmul(ps[:], cw3_sb[:, k, :], rhs, start=(k == 0),
                                     stop=(k == 7))
                nc.scalar.activation(y3[:, grp * 4 * 107:(grp + 1) * 4 * 107], ps[:],
                                     ACT.Relu, bias=cb["cb3"][:, :1])
            yp = cny.tile([128, GPC * 33], BF16, tag="yp", name="yp")
            st["yp"] = yp
            for grp in range(8):
                ps = cnp.tile([128, 4 * 100], F32, tag="pc0", name="pc0")
                for k in range(8):
                    rhs = y3[:].rearrange("p (b t) -> p b t", t=107)[
                        :, grp * 4:(grp + 1) * 4, k:k + 100]
                    nc.tensor.matmul(ps[:], cw4_sb[:, k, :], rhs, start=(k == 0),
                                     stop=(k == 7))
                psr = ps[:].rearrange("p (b t) -> p b t", b=4)
                mx = cn.tile([128, 4 * 33], F32, tag="mx", name="mx")
                mxr = mx[:].rearrange("p (b t) -> p b t", b=4)
                nc.vector.tensor_copy(out=mxr, in_=psr[:, :, 0:99:3])
                nc.vector.tensor_tensor(out=mxr, in0=mxr, in1=psr[:, :, 1:100:3],
                                        op=OP.max)
                nc.vector.tensor_tensor(out=mxr, in0=mxr, in1=psr[:, :, 2:100:3],
                                        op=OP.max)
                nc.scalar.activation(yp[:, grp * 4 * 33:(grp + 1) * 4 * 33], mx[:],
                                     ACT.Relu, bias=cb["cb4"][:, :1])

    def stage3():
        cn, cny = st["cn"], st["cny"]
        yp = st["yp"]
        with tc.tile_pool(name="cnp3", bufs=2, space="PSUM") as cnp:
            xt1 = cny.tile([GPC, 1024], F32, tag="xt1", name="xt1")
            psn = [cnp.tile([GPC, 512], F32, tag=f"pc0_{n}", name=f"pc0_{n}",
                            bufs=1) for n in range(2)]
            for tg in range(9):
                t0_, t1_ = tg * 4, min(tg * 4 + 4, 33)
                w = cny.tile([128, t1_ - t0_, 1024], BF16, tag="fx1w",
                             name="fx1w", bufs=3)
                nc.sync.dma_start(out=w[:], in_=p.w1xt[t0_:t1_].rearrange(
                    "t p n -> p t n"))
                for t_ in range(t0_, t1_):
                    lhs = yp[:].rearrange("p (b t) -> p t b", t=33)[:, t_, :]
                    for n in range(2):
                        nc.tensor.matmul(
                            psn[n][:], lhs,
                            w[:, t_ - t0_, n * 512:(n + 1) * 512],
                            start=(t_ == 0), stop=(t_ == 32))
            for n in range(2):
                nc.vector.tensor_copy(out=xt1[:, n * 512:(n + 1) * 512],
                                      in_=psn[n][:])
            bb = cn.tile([GPC, 1024], F32, tag="fxbb", name="fxbb", bufs=1)
            nc.sync.dma_start(out=bb[:], in_=p.fc1_xt_b[:])
            nc.vector.tensor_tensor(out=xt1[:], in0=xt1[:], in1=bb[:], op=OP.add)
            nc.scalar.activation(xt1[:], xt1[:], ACT.Relu)
            xt1T = [cny.tile([128, GPC], F32, tag=f"xt1T{j}", name=f"xt1T{j}",
                             bufs=1)
                    for j in range(8)]
            for j in range(8):
                _dve_T(nc, xt1T[j], xt1[:, j * 128:(j + 1) * 128], 128)
            ps = cnp.tile([GPC, 128], F32, tag="pc0", name="pc0")
            w8 = cny.tile([128, 8, 128], F32, tag="fx2w", name="fx2w", bufs=1)
            nc.sync.dma_start(out=w8[:], in_=p.fc2_xt_w[:].rearrange(
                "(j p) n -> p j n", p=128))
            for j in range(8):
                nc.tensor.matmul(ps[:], xt1T[j][:], w8[:, j, :], start=(j == 0),
                                 stop=(j == 7))
            p.xt2 = p.head_pool.tile([GPC, 128], F32, tag="xt2", name="xt2")
            bb2 = cn.tile([GPC, 128], F32, tag="fxbb2", name="fxbb2", bufs=1)
            nc.sync.dma_start(out=bb2[:], in_=p.fc2_xt_b[:])
            nc.vector.tensor_tensor(out=p.xt2[:], in0=ps[:], in1=bb2[:], op=OP.add)
            t = tap("xt2", [GPC, 128])
            if t is not None:
                nc.sync.dma_start(out=t[:], in_=p.xt2[:])
        # whole CNN branch done -- release all its pools (LIFO: cny, cnw, cn)
        while st["cms"]:
            st["cms"].pop(0).__exit__(None, None, None)

    return {"stage1_open": stage1_open, "stage1_grp": stage1_grp,
            "stage1_close": stage1_close, "stage2": stage2, "stage3": stage3,
            "cleanup_cm": _Cleanup()}


# ---------------- fusion ----------------
def _fusion(p, tap):
    nc, tc = p.nc, p.tc
    _graph_head(p, tap)
    with (
        tc.tile_pool(name="fu", bufs=2) as fu,
        tc.tile_pool(name="fup", bufs=2, space="PSUM") as fup,
    ):
        xcT = []
        for src_ in (p.g2, p.xt2):
            t = fu.tile([128, GPC], F32, tag=f"xcT{len(xcT)}", name=f"xcT{len(xcT)}")
            _dve_T(nc, t, src_[:], 128)
            xcT.append(t)
        c1 = fu.tile([GPC, 1024], F32, tag="c1", name="c1")
        w2 = fu.tile([128, 2, 1024], F32, tag="f1w", name="f1w")
        nc.sync.dma_start(out=w2[:], in_=p.fc1_w[:].rearrange(
            "(j p) n -> p j n", p=128))
        for n in range(2):
            ps = fup.tile([GPC, 512], F32, tag="mm", name="mm")
            for j in range(2):
                nc.tensor.matmul(ps[:], xcT[j][:],
                                 w2[:, j, n * 512:(n + 1) * 512],
                                 start=(j == 0), stop=(j == 1))
            nc.vector.tensor_copy(out=c1[:, n * 512:(n + 1) * 512], in_=ps[:])
        bb = fu.tile([GPC, 1024], F32, tag="fbb", name="fbb")
        nc.sync.dma_start(out=bb[:], in_=p.fc1_b[:])
        nc.vector.tensor_tensor(out=c1[:], in0=c1[:], in1=bb[:], op=OP.add)
        c1b = fu.tile([GPC, 1024], F32, tag="c1b", name="c1b")
        nc.scalar.activation(c1b[:], c1[:], ACT.Relu)
        c1T = [fu.tile([128, GPC], F32, tag=f"c1T{j}", name=f"c1T{j}") for j in range(8)]
        for j in range(8):
            _dve_T(nc, c1T[j], c1b[:, j * 128:(j + 1) * 128], 128)
        ps = fup.tile([GPC, 256], F32, tag="mm", name="mm")
        wf2 = fu.tile([128, 8, 256], F32, tag="f2w", name="f2w")
        nc.sync.dma_start(out=wf2[:], in_=p.fc2_w[:].rearrange(
            "(j p) n -> p j n", p=128))
        for j in range(8):
            nc.tensor.matmul(ps[:], c1T[j][:], wf2[:, j, :], start=(j == 0),
                             stop=(j == 7))
        c2 = fu.tile([GPC, 256], F32, tag="c2", name="c2")
        bb2 = fu.tile([GPC, 256], F32, tag="fbb2", name="fbb2")
        nc.sync.dma_start(out=bb2[:], in_=p.fc2_b[:])
        nc.vector.tensor_tensor(out=c2[:], in0=ps[:], in1=bb2[:], op=OP.add)
        c2b = fu.tile([GPC, 256], F32, tag="c2b", name="c2b")
        nc.scalar.activation(c2b[:], c2[:], ACT.Relu)
        c2T = []
        for j in range(2):
            t = fu.tile([128, GPC], F32, tag=f"c2T{j}", name=f"c2T{j}")
            _dve_T(nc, t, c2b[:, j * 128:(j + 1) * 128], 128)
            c2T.append(t)
        ow = fu.tile([128, 2], F32, tag="ow", name="ow")
        for j in range(2):
            nc.sync.dma_start(out=ow[:, j:j + 1], in_=p.out_w[j * 128:(j + 1) * 128, :])
        ps = fup.tile([GPC, 1], F32, tag="mm", name="mm")
        for j in range(2):
            nc.tensor.matmul(ps[:], c2T[j][:], ow[:, j:j + 1],
                             start=(j == 0), stop=(j == 1))
        o = fu.tile([GPC, 1], F32, tag="o", name="o")
        nc.vector.tensor_copy(out=o[:], in_=ps[:])
        nc.sync.dma_start(out=p.out[:], in_=o[:])


# ------------------------------------------------------------------ entry
def _build_and_run(inputs, taps=()):
    T_blocks, in_maps, out_b = _host_prep(inputs)
    nc, p = build_program(T_blocks, taps=taps)
    res = run_bass_kernel_spmd(nc, in_maps, list(range(NCORES)))
    return res, out_b, p


def kernel(**inputs) -> np.ndarray:
    res, out_b, _ = _build_and_run(inputs)
    out = np.concatenate([res.results[c]["out"] for c in range(NCORES)], axis=0)
    return (out + out_b).astype(np.float32)


# revision 48
# speedup vs baseline: 1.0258x; 1.0111x over previous
"""GATNet (3x GATConv graph branch + 1D-CNN protein branch + fusion MLP) on 8
Trainium2 NeuronCores via Bass/Tile.

Sharding: nodes row-sharded 1280/core (= 32 graphs/core since batch is sorted
blocks of 40); CNN branch sharded by the same 32 samples/core; weights
replicated in bf16.

Per GAT layer l:
  1. h = x @ [W | W@as_blk | W@ad_blk]  (m-outer node-block loop; attention
     scalars appear as extra columns). Augmented rows (h | a_s as f32
     bitcast | const 1) are written to local DRAM block by block.
  2. Chunked AllGather: every 2 node blocks fire their own AllGather into a
     block-major shared h_full, overlapping collectives with the remaining
     x@W compute. Gather indices are host-remapped to the chunked layout.
  3. Per 128-dst block: dst-sorted edge tiles; indirect-DMA gathers src rows
     into one batched tile; block-batched attention math (one add / Prelu /
     Exp chain over all tiles); numerator+denominator via one-hot S matmuls
     (S pre-scaled by exp for H<=2 layers, features pre-scaled for the
     10-head layer); scale by reciprocal; transpose tiles; bias+activation
     on transposed tiles -> next lhsT.

Self-contained: hardcodes all shapes; builds the per-call edge structure into
the traced program, compiles and runs via run_bass_kernel_spmd.
"""
import numpy as np
import ml_dtypes

import concourse.bass as bass
import concourse.mybir as mybir
import concourse.tile as tile
from concourse.bass_utils import run_bass_kernel_spmd
from concourse.masks import make_identity
from concourse.tile import add_dep_helper

NCORES = 8
N_NODES = 10240
N_GRAPHS = 256
NPC = N_NODES // NCORES          # 1280 nodes/core
GPC = N_GRAPHS // NCORES         # 32 graphs/core
NPG = N_NODES // N_GRAPHS        # 40 nodes/graph
BPC = NPC // 128                 # 10 dst blocks/core
# AllGather chunking per layer: lists of node-block counts per collective.
# L1's x@W is tiny, so one collective; deeper layers pipeline more chunks.
AG_CHUNKS = [[10], [4, 3, 3], [2, 2, 2, 2, 2]]
SEQ = 1000
VOCAB = 26
EMB = 128
NEG_SLOPE = 0.2

F32 = mybir.dt.float32
F32R = mybir.dt.float32r
BF16 = mybir.dt.bfloat16
I32 = mybir.dt.int32
AX = mybir.AxisListType
OP = mybir.AluOpType
ACT = mybir.ActivationFunctionType

# (F_in, F_out, heads)
LAYERS = [(78, 780, 10), (780, 1560, 2), (1560, 3120, 1)]
# x@W psum chunk lists (cover F_out + 2H cols; head-aligned where needed)
MM_CHUNKS = [[390, 390, 20], [390, 390, 390, 390, 4], [448] * 6 + [434]]
# message-pass numerator chunks: per-head col ranges for S-scaled layers
MP_HEAD_CHUNKS = {1: [(0, 512), (512, 780)],
                  2: [(0, 512), (512, 1024), (1024, 1536), (1536, 2048),
                      (2048, 2560), (2560, 3072), (3072, 3120)]}

bf = lambda a: np.ascontiguousarray(a).astype(ml_dtypes.bfloat16)
f32 = lambda a: np.ascontiguousarray(a, dtype=np.float32)
cdiv = lambda a, b: -(-a // b)


# ------------------------------------------------------------------ walrus patch
def _split_sync_waits(nc, max_keep=1):
    for f in nc.m.functions:
        for bb in f.blocks:
            out, changed = [], False
            for ins in bb.instructions:
                si = ins.sync_info
                waits = list(si.on_wait) if si is not None and si.on_wait else []
                if len(waits) > max_keep:
                    extra, keep = waits[:-max_keep], waits[-max_keep:]
                    for i in range(0, len(extra), max_keep):
                        out.append(mybir.InstNoOp(
                            name=f"WSPLIT-{nc.next_id()}", engine=ins.engine,
                            bass_nofuse=True,
                            sync_info=mybir.SyncInfo(on_wait=extra[i:i + max_keep],
                                                     on_update=[])))
                    si.on_wait = keep
                    changed = True
                out.append(ins)
            if changed:
                bb.instructions[:] = out


# ------------------------------------------------------------------ host prep
def _remap_rows(n, li):
    """Global node id -> row in layer li's chunk-major h_full layout."""
    c, j = n // NPC, n % NPC
    m = j // 128
    b0s = np.cumsum([0] + AG_CHUNKS[li][:-1])
    q = np.searchsorted(b0s, m, side="right") - 1
    b0 = b0s[q]
    nb = np.asarray(AG_CHUNKS[li])[q]
    return b0 * (NCORES * 128) + c * (nb * 128) + (j - b0 * 128)


def _edge_structure(edge_index):
    src, dst = edge_index[0].astype(np.int64), edge_index[1].astype(np.int64)
    loop = np.arange(N_NODES, dtype=np.int64)
    s_all = np.concatenate([src, loop])
    d_all = np.concatenate([dst, loop])
    order = np.argsort(d_all, kind="stable")
    s_s, d_s = s_all[order], d_all[order]

    bounds = np.searchsorted(d_s, np.arange(0, N_NODES + 1, 128))
    cnt = bounds[1:] - bounds[:-1]
    tiles_needed = -(-cnt // 128)
    T_blocks = [int(tiles_needed.reshape(NCORES, BPC)[:, p].max()) for p in range(BPC)]
    t_off = np.cumsum([0] + T_blocks)
    T_tot = int(t_off[-1])

    src_idx = np.zeros((NCORES, 3, T_tot, 128), np.int32)
    S = np.zeros((NCORES, T_tot, 128, 128), np.float32)
    for c in range(NCORES):
        for p_ in range(BPC):
            blk = c * BPC + p_
            e0, e1 = int(bounds[blk]), int(bounds[blk + 1])
            m = e1 - e0
            ti = np.arange(m) // 128 + t_off[p_]
            ei = np.arange(m) % 128
            for li in range(3):
                src_idx[c, li, ti, ei] = _remap_rows(s_s[e0:e1], li)
            S[c, ti, ei, d_s[e0:e1] - 128 * blk] = 1.0
    ST = np.swapaxes(S, 2, 3)
    SS = np.concatenate([S, ST], axis=3)          # [8, T, 128, 256]
    # [8, 128, 3*T_tot]
    src_idxT = np.ascontiguousarray(
        np.swapaxes(src_idx.reshape(NCORES, 3 * T_tot, 128), 1, 2))
    return T_blocks, src_idxT, bf(SS)


def _aug_w(W, a_s, a_d, H):
    """[W | W@as_blk | W@ad_blk] with as_blk[f,h] = a_s[h, f - h*FH]."""
    fi, fo = W.shape
    FH = fo // H
    was = np.zeros((fi, H), np.float32)
    wad = np.zeros((fi, H), np.float32)
    for h in range(H):
        was[:, h] = W[:, h * FH:(h + 1) * FH] @ a_s[h]
        wad[:, h] = W[:, h * FH:(h + 1) * FH] @ a_d[h]
    return np.concatenate([W, was, wad], axis=1)


def _bias_colmajor(b, fo):
    n_t = cdiv(fo, 128)
    pad = np.zeros(n_t * 128, np.float32)
    pad[:fo] = b
    return np.ascontiguousarray(pad.reshape(n_t, 128).T)   # [128, n_t]


def _host_prep(inputs):
    ii = {k: np.asarray(v) for k, v in inputs.items()}
    T_blocks, src_idxT, SS = _edge_structure(ii["edge_index"])

    xT = np.ascontiguousarray(np.swapaxes(f32(ii["x"]), 0, 1))   # [78, 10240]

    W_aug, b_col = [], []
    for i, (fi, fo, H) in enumerate(LAYERS):
        W_aug.append(bf(_aug_w(f32(ii[f"W{i+1}"]), f32(ii[f"as{i+1}"]),
                               f32(ii[f"ad{i+1}"]), H)))
        b_col.append(_bias_colmajor(f32(ii[f"b{i+1}"]).reshape(-1), fo))

    cw1 = f32(ii["cw1"])
    cw1f = np.zeros((125, 8, 2, 128), np.float32)
    for sc in range(8):
        for ks in range(2):
            blk = cw1[:, sc * 125:(sc + 1) * 125, ks * 4:(ks + 1) * 4]
            cw1f[:, sc, ks, :] = blk.transpose(1, 2, 0).reshape(125, 128)
    cwT = lambda w: np.ascontiguousarray(np.transpose(f32(ii[w]), (1, 2, 0)))

    w1xt = np.ascontiguousarray(
        f32(ii["fc1_xt_w"]).reshape(128, 33, 1024).transpose(1, 0, 2))

    emb_f = f32(ii["emb_xt"])                                   # [26, 128]
    rep = lambda a, n: np.ascontiguousarray(
        np.broadcast_to(f32(a).reshape(1, -1), (n, f32(a).size)))

    shared = {
        "W1": W_aug[0], "W2": W_aug[1], "W3": W_aug[2],
        "bc1": b_col[0], "bc2": b_col[1], "bc3": b_col[2],
        "fc_g1_w": bf(ii["fc_g1_w"]), "fc_g1_b": rep(ii["fc_g1_b"], GPC),
        "fc_g2_w": f32(ii["fc_g2_w"]), "fc_g2_b": rep(ii["fc_g2_b"], GPC),
        "cw1f": bf(cw1f), "cb1": f32(ii["cb1"]).reshape(-1, 1),
        "cw2T": bf(cwT("cw2")), "cb2": f32(ii["cb2"]).reshape(-1, 1),
        "cw3T": bf(cwT("cw3")), "cb3": f32(ii["cb3"]).reshape(-1, 1),
        "cw4T": bf(cwT("cw4")), "cb4": f32(ii["cb4"]).reshape(-1, 1),
        "w1xt": bf(w1xt), "fc1_xt_b": rep(ii["fc1_xt_b"], GPC),
        "fc2_xt_w": f32(ii["fc2_xt_w"]), "fc2_xt_b": rep(ii["fc2_xt_b"], GPC),
        "fc1_w": f32(ii["fc1_w"]), "fc1_b": rep(ii["fc1_b"], GPC),
        "fc2_w": f32(ii["fc2_w"]), "fc2_b": rep(ii["fc2_b"], GPC),
        "out_w": f32(ii["out_w"]),
    }
    in_maps = []
    for c in range(NCORES):
        m = dict(shared)
        m["xT"] = bf(xT[:, c * NPC:(c + 1) * NPC])
        m["esrcT"] = src_idxT[c]
        m["SS"] = SS[c]
        tgt_c = ii["target"][c * GPC:(c + 1) * GPC]               # [32, 1000]
        E_full = emb_f[tgt_c]                                     # [32, 1000, 128]
        E2 = E_full.reshape(8, 4, 8, 125, 128).transpose(2, 0, 3, 1, 4)
        m["E2"] = bf(E2.reshape(8, 8, 125, 512))
        in_maps.append(m)
    out_b = float(np.asarray(ii["out_b"]).reshape(-1)[0])
    return T_blocks, in_maps, out_b


# ------------------------------------------------------------------ program
class P:
    pass


def _aug_cols(li):
    fo, H = LAYERS[li][1], LAYERS[li][2]
    return fo + 2 * H + 2        # h | a_s(f32 as 2H bf16) | ones | pad


def build_program(T_blocks, taps=()):
    T_tot = sum(T_blocks)
    nc = bass.Bass()
    p = P()
    p.nc = nc
    p.taps = set(taps)
    p.tap_tensors = {}

    dp = lambda name, shape, dt: nc.declare_dram_parameter(name, list(shape), dt,
                                                           isOutput=False)
    p.xT = dp("xT", [78, NPC], BF16)
    p.W = [dp(f"W{i+1}", [LAYERS[i][0], LAYERS[i][1] + 2 * LAYERS[i][2]], BF16)
           for i in range(3)]
    p.bc = [dp(f"bc{i+1}", [128, cdiv(LAYERS[i][1], 128)], F32) for i in range(3)]
    p.esrcT = dp("esrcT", [128, 3 * T_tot], I32)
    p.SS = dp("SS", [T_tot, 128, 256], BF16)
    p.fc_g1_w = dp("fc_g1_w", [3120, 1024], BF16)
    p.fc_g1_b = dp("fc_g1_b", [GPC, 1024], F32)
    p.fc_g2_w = dp("fc_g2_w", [1024, 128], F32)
    p.fc_g2_b = dp("fc_g2_b", [GPC, 128], F32)
    p.E2 = dp("E2", [8, 8, 125, 512], BF16)
    p.cw1f = dp("cw1f", [125, 8, 2, 128], BF16)
    p.cb1 = dp("cb1", [32, 1], F32)
    p.cw2T = dp("cw2T", [32, 8, 64], BF16)
    p.cb2 = dp("cb2", [64, 1], F32)
    p.cw3T = dp("cw3T", [64, 8, 96], BF16)
    p.cb3 = dp("cb3", [96, 1], F32)
    p.cw4T = dp("cw4T", [96, 8, 128], BF16)
    p.cb4 = dp("cb4", [128, 1], F32)
    p.w1xt = dp("w1xt", [33, 128, 1024], BF16)
    p.fc1_xt_b = dp("fc1_xt_b", [GPC, 1024], F32)
    p.fc2_xt_w = dp("fc2_xt_w", [1024, 128], F32)
    p.fc2_xt_b = dp("fc2_xt_b", [GPC, 128], F32)
    p.fc1_w = dp("fc1_w", [256, 1024], F32)
    p.fc1_b = dp("fc1_b", [GPC, 1024], F32)
    p.fc2_w = dp("fc2_w", [1024, 256], F32)
    p.fc2_b = dp("fc2_b", [GPC, 256], F32)
    p.out_w = dp("out_w", [256, 1], F32)
    p.out = nc.declare_dram_parameter("out", [GPC, 1], F32, isOutput=True)

    p.h_loc = [nc.dram_tensor(f"h{i+1}_loc", [NPC, _aug_cols(i)], BF16)
               for i in range(3)]
    p.h_full = [nc.dram_tensor(f"h{i+1}_full", [N_NODES, _aug_cols(i)], BF16,
                               addr_space="Shared") for i in range(3)]

    def tap(name, shape, dt=F32):
        if name in p.taps:
            t = nc.declare_dram_parameter("tap_" + name, list(shape), dt,
                                          isOutput=True)
            p.tap_tensors[name] = t
            return t
        return None

    with tile.TileContext(nc) as tc:
        p.tc = tc
        _cp_cm = tc.tile_pool(name="const", bufs=1)
        const_pool = _cp_cm.__enter__()
        p.ident = const_pool.tile([128, 128], BF16)
        make_identity(nc, p.ident[:])
        p.ones_col = const_pool.tile([128, 1], BF16, tag="ones1", name="ones1")
        nc.vector.memset(p.ones_col[:], 1.0)
        p.join_dummy = const_pool.tile([1, 1], F32, tag="jd", name="jd")
        p.head_pool = const_pool

        stages = _cnn_make(p, tap)
        p.cnn_stages = stages
        _gat_branch(p, T_blocks, tap)
        _fusion(p, tap)
        for cm in p.gat_cleanup:
            cm.__exit__(None, None, None)
        _cp_cm.__exit__(None, None, None)

    _split_sync_waits(nc)
    return nc, p


# ---------------- GAT branch ----------------
def _gat_branch(p, T_blocks, tap):
    nc, tc = p.nc, p.tc

    mpc_cm = tc.tile_pool(name="mpc", bufs=1)
    mpc_pool = mpc_cm.__enter__()
    eidx = mpc_pool.tile([128, 3 * sum(T_blocks)], I32, tag="eidx", name="eidx")
    nc.sync.dma_start(out=eidx[:], in_=p.esrcT[:])
    p.eidx = eidx
    adp_cms = [tc.tile_pool(name=f"adp{li}", bufs=1) for li in range(3)]
    adp_pools = [cm.__enter__() for cm in adp_cms]

    xT_cm = tc.tile_pool(name="xT0", bufs=1)
    xT_pool = xT_cm.__enter__()
    xT_tiles = [xT_pool.tile([78, NPC], BF16, tag="x0", name="x0")]
    nc.sync.dma_start(out=xT_tiles[0][:], in_=p.xT[:])

    for li, (fi, fo, H) in enumerate(LAYERS):
        is_last = li == 2
        n_k = cdiv(fi, 128)
        cols = _aug_cols(li)
        a_d_pool = adp_pools[li]
        a_d_tiles = []
        chunks = MM_CHUNKS[li]
        offs = [int(v) for v in np.cumsum([0] + chunks)]
        n_ch = len(chunks)
        ag_insts = []
        with (
            tc.tile_pool(name=f"w{li}", bufs=1) as wpool,
            tc.tile_pool(name=f"mm{li}", bufs=2) as mpool,
            tc.tile_pool(name=f"mmp{li}", bufs=2 if li == 0 else 1,
                         space="PSUM") as pspool,
        ):
            W_sb = []
            for k in range(n_k):
                kp = min(128, fi - k * 128)
                t = wpool.tile([kp, offs[-1]], BF16, tag=f"W{k}", name=f"W{k}")
                nc.sync.dma_start(out=t[:], in_=p.W[li][k * 128:k * 128 + kp, :])
                W_sb.append(t)
            pending_writes = []
            for m in range(BPC):
                psums = [pspool.tile([128, chunks[n]], F32, tag=f"hp{n}",
                                     name=f"hp{n}")
                         for n in range(n_ch)]
                for k in range(n_k):
                    kp = min(128, fi - k * 128)
                    lhs = xT_tiles[k][:kp, m * 128:(m + 1) * 128]
                    for n in range(n_ch):
                        nc.tensor.matmul(
                            psums[n][:], lhs, W_sb[k][:, offs[n]:offs[n + 1]],
                            start=(k == 0), stop=(k == n_k - 1))
                stage = mpool.tile([128, cols], BF16, tag="stage", name="stage")
                for n in range(n_ch):
                    lo, hi = offs[n], offs[n + 1]
                    if hi <= fo:
                        nc.scalar.copy(out=stage[:, lo:hi], in_=psums[n][:])
                    else:
                        if lo < fo:
                            nc.scalar.copy(out=stage[:, lo:fo],
                                           in_=psums[n][:, :fo - lo])
                        a_sf = mpool.tile([128, H], F32, tag="a_sf", name="a_sf")
                        nc.vector.tensor_copy(
                            out=a_sf[:], in_=psums[n][:, fo - lo:fo - lo + H])
                        a_d = a_d_pool.tile([128, H], BF16, tag=f"a_d{m}",
                                            name=f"a_d{m}")
                        nc.vector.tensor_copy(
                            out=a_d[:],
                            in_=psums[n][:, fo - lo + H:fo - lo + 2 * H])
                        a_d_tiles.append(a_d)
                        nc.vector.tensor_copy(
                            out=stage[:, fo:fo + 2 * H],
                            in_=a_sf[:].bitcast(BF16))
                oc = fo + 2 * H
                nc.vector.memset(stage[:, oc:oc + 1], 1.0)
                nc.vector.memset(stage[:, oc + 1:cols], 0.0)
                w = nc.scalar.dma_start(
                    out=p.h_loc[li][m * 128:(m + 1) * 128, :], in_=stage[:])
                pending_writes.append(w)
                b0s = [int(v) for v in np.cumsum([0] + AG_CHUNKS[li][:-1])]
                if m + 1 - b0s[len(ag_insts)] == AG_CHUNKS[li][len(ag_insts)]:
                    b0, nb = b0s[len(ag_insts)], AG_CHUNKS[li][len(ag_insts)]
                    cc = nc.gpsimd.collective_compute(
                        "AllGather", OP.bypass,
                        replica_groups=[list(range(NCORES))],
                        ins=[p.h_loc[li][b0 * 128:(b0 + nb) * 128, :]],
                        outs=[p.h_full[li][b0 * NCORES * 128:
                                           (b0 + nb) * NCORES * 128, :]])
                    for w_ in pending_writes:
                        add_dep_helper(cc.ins, w_.ins, reason="AG waits h_loc")
                    pending_writes = []
                    ag_insts.append(cc)

        # join: one gpsimd op that waits for all AG chunks of this layer
        join = nc.gpsimd.memset(p.join_dummy[:], float(li))
        for cc in ag_insts:
            add_dep_helper(join.ins, cc.ins, reason="join waits AG chunk")

        t = tap(f"h{li+1}", [NPC, cols], BF16)
        if t is not None:
            d = nc.sync.dma_start(out=t[:], in_=p.h_loc[li][:])
            add_dep_helper(d.ins, join.ins, reason="tap waits AGs")

        xT_cm.__exit__(None, None, None)

        filler = None
        if li == 0:
            p.cnn_stages["stage1_open"]()

            def filler(blk):
                if blk < 8:
                    p.cnn_stages["stage1_grp"](blk)
                elif blk == 8:
                    p.cnn_stages["stage1_close"]()
        elif li == 1:
            p.cnn_stages["stage2"]()
        elif li == 2:
            p.cnn_stages["stage3"]()

        n_kT = cdiv(fo, 128)
        xTn_cm = tc.tile_pool(name=f"xTn{li}", bufs=1)
        xTn_pool = xTn_cm.__enter__()
        xT_out = []
        for j in range(n_kT):
            kp = min(128, fo - j * 128)
            xT_out.append(xTn_pool.tile([kp, NPC], BF16, tag=f"xT{li}_{j}",
                                        name=f"xT{li}_{j}"))

        _message_pass(p, T_blocks, li, a_d_tiles, join, xT_out, filler)

        t = tap(f"xT{li+2}" if not is_last else "o3T", [fo, NPC], BF16)
        if t is not None:
            for j in range(n_kT):
                kp = min(128, fo - j * 128)
                nc.sync.dma_start(out=t[j * 128:j * 128 + kp, :], in_=xT_out[j][:])

        xT_tiles = xT_out
        xT_cm = xTn_cm
        if is_last:
            p.out3T = xT_out
            p.gat_cleanup = [xTn_cm, p.cnn_stages["cleanup_cm"],
                             *reversed(adp_cms), mpc_cm]
    return


def _message_pass(p, T_blocks, li, a_d_tiles, ag_join, xT_out, filler=None):
    nc, tc = p.nc, p.tc
    fi, fo, H = LAYERS[li]
    FH = fo // H
    cols = _aug_cols(li)
    t_off = np.cumsum([0] + T_blocks)
    n_kT = cdiv(fo, 128)
    s_scale = li > 0                 # scale S by exp for H<=2; scale g for H=10
    nsub = 1 if li == 0 else 2

    with (
        tc.tile_pool(name=f"mp{li}", bufs=2) as mp,
        tc.tile_pool(name=f"mpS{li}", bufs=3) as mpS,
        tc.tile_pool(name=f"bc{li}", bufs=1) as bcp,
    ):
        bcol = bcp.tile([128, n_kT], F32, tag="bcol", name="bcol")
        nc.sync.dma_start(out=bcol[:], in_=p.bc[li][:])

        for blk in range(BPC):
            if filler is not None:
                filler(blk)
            Tb = T_blocks[blk]
            t0 = int(t_off[blk])
            e0 = li * int(t_off[-1]) + t0
            subs = [(0, Tb)] if nsub == 1 else \
                [(0, (Tb + 1) // 2), ((Tb + 1) // 2, Tb - (Tb + 1) // 2)]
            rows_bf = mp.tile([128, fo], BF16, tag="rows_bf", name="rows_bf")
            rec = mp.tile([128, H], F32, tag="rec", name="rec")
            with (
                tc.tile_pool(name=f"op{li}_{blk}", bufs=1, space="PSUM") as pp,
            ):
                if s_scale:
                    head_chunks = MP_HEAD_CHUNKS[li]
                    n_hc = len(head_chunks)
                    dlast = head_chunks[-1][1] - head_chunks[-1][0]
                    opsum = {}
                    for h in range(H):
                        for ci, (lo, hi) in enumerate(head_chunks):
                            w_ = hi - lo + (1 if ci == n_hc - 1 else 0)
                            opsum[(h, ci)] = pp.tile(
                                [128, w_], F32,
                                tag=f"op{h}_{ci}", name=f"op{h}_{ci}")
                else:
                    num0 = pp.tile([128, 390], F32, tag="num0", name="num0")
                    num1 = pp.tile([128, 390], F32, tag="num1", name="num1")
                    dn = pp.tile([128, H], F32, tag="dn", name="dn")
                aux = pp.tile([128, Tb * H], F32, tag="aux", name="aux")

                for si, (ta, tn) in enumerate(subs):
                    is_fs, is_ls = si == 0, si == nsub - 1
                    SS_sub = mpS.tile([128, tn, 256], BF16, tag=f"SS{si}",
                                      name=f"SS{si}")
                    nc.sync.dma_start(
                        out=SS_sub[:],
                        in_=p.SS[t0 + ta:t0 + ta + tn].rearrange(
                            "t p c -> p t c"))
                    g_sub = mp.tile([128, tn, cols], BF16, tag=f"g{si}",
                                    name=f"g{si}", bufs=3)
                    for t in range(tn):
                        gi = nc.gpsimd.indirect_dma_start(
                            out=g_sub[:, t, :], out_offset=None,
                            in_=p.h_full[li][:],
                            in_offset=bass.IndirectOffsetOnAxis(
                                ap=p.eidx[:, e0 + ta + t:e0 + ta + t + 1],
                                axis=0))
                        add_dep_helper(gi.ins, ag_join.ins,
                                       reason="gather waits AG")
                    for t in range(tn):
                        nc.tensor.matmul(
                            aux[:, (ta + t) * H:(ta + t + 1) * H],
                            SS_sub[:, t, 128:256],
                            a_d_tiles[blk][:], start=True, stop=True)
                    sc = mp.tile([128, tn * H], F32, tag=f"sc{si}",
                                 name=f"sc{si}")
                    nc.vector.tensor_tensor(
                        out=sc[:].rearrange("p (t h) -> p t h", h=H),
                        in0=g_sub[:, :, fo:fo + 2 * H].bitcast(F32),
                        in1=aux[:, ta * H:(ta + tn) * H].rearrange(
                            "p (t h) -> p t h", h=H),
                        op=OP.add)
                    nc.scalar.activation(sc[:], sc[:], ACT.Prelu,
                                         alpha=NEG_SLOPE)
                    ex = mp.tile([128, tn * H], BF16, tag=f"ex{si}",
                                 name=f"ex{si}")
                    nc.scalar.activation(ex[:], sc[:], ACT.Exp)
                    ex3 = ex[:].rearrange("p (t h) -> p t h", h=H)

                    if s_scale:
                        Ssc = []
                        for h in range(H):
                            sh = mp.tile([128, tn * 128], BF16,
                                         tag=f"Ssc{si}_{h}", name=f"Ssc{si}_{h}")
                            nc.vector.tensor_tensor(
                                out=sh[:].rearrange("p (t c) -> p t c", c=128),
                                in0=SS_sub[:, :, 0:128],
                                in1=ex3[:, :, h:h + 1].broadcast_to(
                                    [128, tn, 128]),
                                op=OP.mult)
                            Ssc.append(sh)
                        # denominators first (own their bank's initial clear)
                        for h in range(H):
                            for t in range(tn):
                                nc.tensor.matmul(
                                    opsum[(h, n_hc - 1)][:, dlast:dlast + 1],
                                    Ssc[h][:, t * 128:(t + 1) * 128],
                                    p.ones_col[:],
                                    start=(is_fs and t == 0),
                                    stop=(is_ls and t == tn - 1),
                                    skip_group_check=not is_fs)
                        if is_ls:
                            for h in range(H):
                                nc.vector.tensor_scalar(
                                    out=rec[:, h:h + 1],
                                    in0=opsum[(h, n_hc - 1)][:,
                                              dlast:dlast + 1],
                                    scalar1=1e-16, scalar2=None, op0=OP.add)
                            nc.vector.reciprocal(rec[:], rec[:])
                        # numerators chunk-major within this sub-unit; the
                        # last chunk's bank was cleared by the denominator
                        # group -> start=False overwrite-on-cleared
                        for ci, (lo, hi) in enumerate(head_chunks):
                            is_dl = ci == n_hc - 1
                            for h in range(H):
                                for t in range(tn):
                                    nc.tensor.matmul(
                                        opsum[(h, ci)][:, :hi - lo],
                                        Ssc[h][:, t * 128:(t + 1) * 128],
                                        g_sub[:, t, h * FH + lo:h * FH + hi],
                                        start=(is_fs and t == 0 and not is_dl),
                                        stop=(is_ls and t == tn - 1),
                                        skip_group_check=is_dl or not is_fs)
                                if is_ls:
                                    nc.vector.tensor_scalar(
                                        out=rows_bf[:,
                                                    h * FH + lo:h * FH + hi],
                                        in0=opsum[(h, ci)][:, :hi - lo],
                                        scalar1=rec[:, h:h + 1], scalar2=None,
                                        op0=OP.mult)
                    else:
                        # one 4D broadcast multiply scales all heads at once
                        gs = mp.tile([128, tn, fo + H], BF16, tag=f"gs{si}",
                                     name=f"gs{si}", bufs=3)
                        nc.vector.tensor_tensor(
                            out=gs[:, :, 0:fo].rearrange(
                                "p t (h f) -> p t h f", f=FH),
                            in0=g_sub[:, :, 0:fo].rearrange(
                                "p t (h f) -> p t h f", f=FH),
                            in1=ex3.unsqueeze(3).broadcast_to(
                                [128, tn, H, FH]),
                            op=OP.mult)
                        nc.vector.tensor_copy(out=gs[:, :, fo:fo + H], in_=ex3)
                        for t in range(tn):
                            nc.tensor.matmul(
                                dn[:], SS_sub[:, t, 0:128],
                                gs[:, t, fo:fo + H],
                                start=(is_fs and t == 0),
                                stop=(is_ls and t == tn - 1))
                        if is_ls:
                            nc.vector.tensor_scalar(
                                out=rec[:], in0=dn[:], scalar1=1e-16,
                                scalar2=None, op0=OP.add)
                            nc.vector.reciprocal(rec[:], rec[:])
                        for ni, (nt, lo, hi) in enumerate(
                                [(None, 0, 390), (None, 390, 780)]):
                            tgt = num0 if ni == 0 else num1
                            for t in range(tn):
                                nc.tensor.matmul(
                                    tgt[:], SS_sub[:, t, 0:128],
                                    gs[:, t, lo:hi],
                                    start=(is_fs and t == 0),
                                    stop=(is_ls and t == tn - 1))
                            if is_ls:
                                nh = 5
                                h0 = 0 if ni == 0 else 5
                                nc.vector.tensor_tensor(
                                    out=rows_bf[:, lo:hi].rearrange(
                                        "p (h f) -> p h f", f=FH),
                                    in0=tgt[:].rearrange(
                                        "p (h f) -> p h f", f=FH),
                                    in1=rec[:, h0:h0 + nh].unsqueeze(
                                        2).broadcast_to([128, nh, FH]),
                                    op=OP.mult)
            if s_scale:
                with tc.tile_pool(name=f"tp{li}_{blk}", bufs=2,
                                  space="PSUM") as ptp:
                    for j in range(n_kT):
                        kp = min(128, fo - j * 128)
                        tp = ptp.tile([kp, 128], BF16, tag="tp", name="tp")
                        nc.tensor.transpose(
                            tp[:], rows_bf[:, j * 128:j * 128 + kp], p.ident[:])
                        nc.scalar.activation(
                            xT_out[j][:, blk * 128:(blk + 1) * 128], tp[:],
                            ACT.Relu, bias=bcol[:kp, j:j + 1])
            else:
                # transpose + batched ELU epilogue
                with tc.tile_pool(name=f"tp{li}_{blk}", bufs=2,
                                  space="PSUM") as ptp:
                    zall = mp.tile([128, n_kT * 128], F32, tag="zall",
                                   name="zall")
                    for j in range(n_kT):
                        kp = min(128, fo - j * 128)
                        tp = ptp.tile([kp, 128], BF16, tag="tp", name="tp")
                        nc.tensor.transpose(
                            tp[:], rows_bf[:, j * 128:j * 128 + kp], p.ident[:])
                        nc.scalar.activation(
                            zall[:kp, j * 128:(j + 1) * 128], tp[:],
                            ACT.Identity, bias=bcol[:kp, j:j + 1])
                    t1 = mp.tile([128, n_kT * 128], F32, tag="elu1", name="elu1")
                    nc.vector.tensor_scalar(out=t1[:], in0=zall[:], scalar1=0.0,
                                            scalar2=None, op0=OP.min)
                    nc.scalar.activation(t1[:], t1[:], ACT.Exp)
                    nc.scalar.activation(zall[:], zall[:], ACT.Relu)
                    for j in range(n_kT):
                        kp = min(128, fo - j * 128)
                        nc.vector.scalar_tensor_tensor(
                            out=xT_out[j][:, blk * 128:(blk + 1) * 128],
                            in0=zall[:kp, j * 128:(j + 1) * 128], scalar=-1.0,
                            in1=t1[:kp, j * 128:(j + 1) * 128],
                            op0=OP.add, op1=OP.add)


def _dve_T(nc, dst, src, n):
    """dst[n, 32] = src[32, n].T via DVE 32x32 block transposes."""
    for i in range(n // 32):
        nc.vector.transpose(out=dst[32 * i:32 * (i + 1), :],
                            in_=src[:, 32 * i:32 * (i + 1)])


# ---------------- graph head ----------------
def _graph_head(p, tap):
    nc, tc = p.nc, p.tc
    n_kT = len(p.out3T)
    with (
        tc.tile_pool(name="gh", bufs=2) as gh,
        tc.tile_pool(name="ghG", bufs=1) as ghG,
        tc.tile_pool(name="ghp", bufs=2, space="PSUM") as ghp,
    ):
        gT = [ghG.tile([min(128, 3120 - j * 128), GPC], BF16, tag=f"gT{j}", name=f"gT{j}")
              for j in range(n_kT)]
        for j in range(n_kT):
            kp = min(128, 3120 - j * 128)
            gm = gh.tile([kp, GPC * 20], BF16, tag="gmx", name="gmx")
            v = p.out3T[j][:].rearrange("p (g n) -> p g n", n=NPG)
            nc.vector.tensor_tensor(
                out=gm[:].rearrange("p (g n) -> p g n", n=20),
                in0=v[:, :, 0:20], in1=v[:, :, 20:40], op=OP.max)
            nc.vector.reduce_max(
                gT[j][:], gm[:].rearrange("p (g n) -> p g n", n=20),
                axis=AX.X)
        g1 = ghG.tile([GPC, 1024], F32, tag="g1", name="g1")
        psn = [ghp.tile([GPC, 512], F32, tag=f"mm{n}", name=f"mm{n}", bufs=1)
               for n in range(2)]
        for j in range(n_kT):
            kp = min(128, 3120 - j * 128)
            w = gh.tile([kp, 1024], BF16, tag="fg1w", name="fg1w", bufs=3)
            nc.sync.dma_start(out=w[:], in_=p.fc_g1_w[j * 128:j * 128 + kp, :])
            for n in range(2):
                nc.tensor.matmul(psn[n][:], gT[j][:],
                                 w[:, n * 512:(n + 1) * 512], start=(j == 0),
                                 stop=(j == n_kT - 1))
        for n in range(2):
            nc.vector.tensor_copy(out=g1[:, n * 512:(n + 1) * 512],
                                  in_=psn[n][:])
        bb1 = gh.tile([GPC, 1024], F32, tag="ghbb", name="ghbb")
        nc.sync.dma_start(out=bb1[:], in_=p.fc_g1_b[:])
        nc.vector.tensor_tensor(out=g1[:], in0=g1[:], in1=bb1[:], op=OP.add)
        g1b = ghG.tile([GPC, 1024], F32, tag="g1b", name="g1b")
        nc.scalar.activation(g1b[:], g1[:], ACT.Relu)
        g1T = [ghG.tile([128, GPC], F32, tag=f"g1T{j}", name=f"g1T{j}") for j in range(8)]
        for j in range(8):
            _dve_T(nc, g1T[j], g1b[:, j * 128:(j + 1) * 128], 128)
        ps = ghp.tile([GPC, 128], F32, tag="mm", name="mm")
        w8 = gh.tile([128, 8, 128], F32, tag="fg2w", name="fg2w")
        nc.sync.dma_start(out=w8[:], in_=p.fc_g2_w[:].rearrange(
            "(j p) n -> p j n", p=128))
        for j in range(8):
            nc.tensor.matmul(ps[:], g1T[j][:], w8[:, j, :], start=(j == 0),
                             stop=(j == 7))
        p.g2 = p.head_pool.tile([GPC, 128], F32, tag="g2", name="g2")
        bb2 = gh.tile([GPC, 128], F32, tag="ghbb2", name="ghbb2")
        nc.sync.dma_start(out=bb2[:], in_=p.fc_g2_b[:])
        nc.vector.tensor_tensor(out=p.g2[:], in0=ps[:], in1=bb2[:], op=OP.add)
        t = tap("g2", [GPC, 128])
        if t is not None:
            nc.sync.dma_start(out=t[:], in_=p.g2[:])


# ---------------- CNN branch ----------------
def _cnn_make(p, tap):
    """CNN branch split into stages so the orchestrator can interleave them
    into the AllGather gaps. Pools open at stage1, closed via cleanup_cm."""
    nc, tc = p.nc, p.tc
    st = {}

    class _Cleanup:
        def __exit__(self, *a):
            for cm in st["cms"]:
                cm.__exit__(None, None, None)

    def stage1_open():
        cn_cm = tc.tile_pool(name="cn", bufs=3)
        cnw_cm = tc.tile_pool(name="cnw", bufs=1)
        cny_cm = tc.tile_pool(name="cny", bufs=1)
        cn = cn_cm.__enter__()
        cnw = cnw_cm.__enter__()
        cny = cny_cm.__enter__()
        st["cms"] = [cny_cm, cnw_cm, cn_cm]
        st["cn"], st["cnw"], st["cny"] = cn, cnw, cny

        cw1f_sb = cny.tile([125, 8, 2, 128], BF16, tag="cw1f", name="cw1f")
        nc.sync.dma_start(out=cw1f_sb[:], in_=p.cw1f[:])
        cw2_sb = cnw.tile([32, 8, 64], BF16, tag="cw2", name="cw2")
        nc.sync.dma_start(out=cw2_sb[:], in_=p.cw2T[:])
        cw3_sb = cnw.tile([64, 8, 96], BF16, tag="cw3", name="cw3")
        nc.sync.dma_start(out=cw3_sb[:], in_=p.cw3T[:])
        cw4_sb = cnw.tile([96, 8, 128], BF16, tag="cw4", name="cw4")
        nc.sync.dma_start(out=cw4_sb[:], in_=p.cw4T[:])
        cb = {}
        for nm, sh in [("cb1", 32), ("cb2", 64), ("cb3", 96), ("cb4", 128)]:
            cb[nm] = cnw.tile([sh, 1], F32, tag=nm, name=nm)
            nc.sync.dma_start(out=cb[nm][:], in_=getattr(p, nm)[:])
        st.update(cw1f=cw1f_sb, cw2=cw2_sb, cw3=cw3_sb, cw4=cw4_sb, cb=cb)
        st["y1"] = cny.tile([32, GPC * 121], BF16, tag="y1", name="y1")
        st["cnp1_cm"] = tc.tile_pool(name="cnp1", bufs=2, space="PSUM")
        st["cnp1"] = st["cnp1_cm"].__enter__()

    def stage1_grp(grp):
        cn, cnp, cb = st["cn"], st["cnp1"], st["cb"]
        cw1f_sb, y1 = st["cw1f"], st["y1"]
        pc = [cnp.tile([128, 512], F32, tag=f"pc{k}", name=f"pc{k}", bufs=1)
              for k in range(2)]
        for sc in range(8):
            E = cn.tile([125, 512], BF16, tag="E", name="E")
            nc.sync.dma_start(out=E[:], in_=p.E2[sc, grp])
            for ks in range(2):
                nc.tensor.matmul(pc[ks][:], cw1f_sb[:, sc, ks, :], E[:],
                                 start=(sc == 0), stop=(sc == 7))
        acc = cn.tile([32, 4 * 121], F32, tag="c1acc", name="c1acc")
        accr = acc[:].rearrange("p (b t) -> p b t", b=4)
        firstop = True
        for ks in range(2):
            for kl in range(4):
                k = ks * 4 + kl
                src = pc[ks][:].rearrange("p (b j) -> p b j", b=4)[
                    kl * 32:(kl + 1) * 32, :, k:k + 121]
                if firstop:
                    nc.vector.tensor_copy(out=accr, in_=src)
                    firstop = False
                else:
                    nc.vector.tensor_tensor(out=accr, in0=accr, in1=src,
                                            op=OP.add)
        nc.scalar.activation(y1[:, grp * 4 * 121:(grp + 1) * 4 * 121],
                             acc[:], ACT.Relu, bias=cb["cb1"][:32, :1])

    def stage1_close():
        st["cnp1_cm"].__exit__(None, None, None)

    def stage2():
        cn, cny, cb = st["cn"], st["cny"], st["cb"]
        cw2_sb, cw3_sb, cw4_sb = st["cw2"], st["cw3"], st["cw4"]
        y1 = st["y1"]
        with tc.tile_pool(name="cnp2", bufs=2, space="PSUM") as cnp:
            y2 = cny.tile([64, GPC * 114], BF16, tag="y2", name="y2")
            for grp in range(8):
                ps = cnp.tile([64, 4 * 114], F32, tag="pc0", name="pc0")
                for k in range(8):
                    rhs = y1[:].rearrange("p (b t) -> p b t", t=121)[
                        :, grp * 4:(grp + 1) * 4, k:k + 114]
                    nc.tensor.matmul(ps[:], cw2_sb[:, k, :], rhs, start=(k == 0),
                                     stop=(k == 7))
                nc.scalar.activation(y2[:, grp * 4 * 114:(grp + 1) * 4 * 114], ps[:],
                                     ACT.Relu, bias=cb["cb2"][:, :1])
            y3 = cny.tile([96, GPC * 107], BF16, tag="y3", name="y3")
            for grp in range(8):
                ps = cnp.tile([96, 4 * 107], F32, tag="pc0", name="pc0")
                for k in range(8):
                    rhs = y2[:].rearrange("p (b t) -> p b t", t=114)[
                        :, grp * 4:(grp + 1) * 4, k:k + 107]
                    nc.tensor.matmul(ps[:], cw3_sb[:, k, :], rhs, start=(k == 0),
                                     stop=(k == 7))
                nc.scalar.activation(y3[:, grp * 4 * 107:(grp + 1) * 4 * 107], ps[:],
                                     ACT.Relu, bias=cb["cb3"][:, :1])
            yp = cny.tile([128, GPC * 33], BF16, tag="yp", name="yp")
            st["yp"] = yp
            for grp in range(8):
                ps = cnp.tile([128, 4 * 100], F32, tag="pc0", name="pc0")
                for k in range(8):
                    rhs = y3[:].rearrange("p (b t) -> p b t", t=107)[
                        :, grp * 4:(grp + 1) * 4, k:k + 100]
                    nc.tensor.matmul(ps[:], cw4_sb[:, k, :], rhs, start=(k == 0),
                                     stop=(k == 7))
                psr = ps[:].rearrange("p (b t) -> p b t", b=4)
                mx = cn.tile([128, 4 * 33], F32, tag="mx", name="mx")
                mxr = mx[:].rearrange("p (b t) -> p b t", b=4)
                nc.vector.tensor_copy(out=mxr, in_=psr[:, :, 0:99:3])
                nc.vector.tensor_tensor(out=mxr, in0=mxr, in1=psr[:, :, 1:100:3],
                                        op=OP.max)
                nc.vector.tensor_tensor(out=mxr, in0=mxr, in1=psr[:, :, 2:100:3],
                                        op=OP.max)
                nc.scalar.activation(yp[:, grp * 4 * 33:(grp + 1) * 4 * 33], mx[:],
                                     ACT.Relu, bias=cb["cb4"][:, :1])

    def stage3():
        cn, cny = st["cn"], st["cny"]
        yp = st["yp"]
        with tc.tile_pool(name="cnp3", bufs=2, space="PSUM") as cnp:
            xt1 = cny.tile([GPC, 1024], F32, tag="xt1", name="xt1")
            psn = [cnp.tile([GPC, 512], F32, tag=f"pc0_{n}", name=f"pc0_{n}",
                            bufs=1) for n in range(2)]
            for tg in range(9):
                t0_, t1_ = tg * 4, min(tg * 4 + 4, 33)
                w = cny.tile([128, t1_ - t0_, 1024], BF16, tag="fx1w",
                             name="fx1w", bufs=3)
                nc.sync.dma_start(out=w[:], in_=p.w1xt[t0_:t1_].rearrange(
                    "t p n -> p t n"))
                for t_ in range(t0_, t1_):
                    lhs = yp[:].rearrange("p (b t) -> p t b", t=33)[:, t_, :]
                    for n in range(2):
                        nc.tensor.matmul(
                            psn[n][:], lhs,
                            w[:, t_ - t0_, n * 512:(n + 1) * 512],
                            start=(t_ == 0), stop=(t_ == 32))
            for n in range(2):
                nc.vector.tensor_copy(out=xt1[:, n * 512:(n + 1) * 512],
                                      in_=psn[n][:])
            bb = cn.tile([GPC, 1024], F32, tag="fxbb", name="fxbb", bufs=1)
            nc.sync.dma_start(out=bb[:], in_=p.fc1_xt_b[:])
            nc.vector.tensor_tensor(out=xt1[:], in0=xt1[:], in1=bb[:], op=OP.add)
            nc.scalar.activation(xt1[:], xt1[:], ACT.Relu)
            xt1T = [cny.tile([128, GPC], F32, tag=f"xt1T{j}", name=f"xt1T{j}",
                             bufs=1)
                    for j in range(8)]
            for j in range(8):
                _dve_T(nc, xt1T[j], xt1[:, j * 128:(j + 1) * 128], 128)
            ps = cnp.tile([GPC, 128], F32, tag="pc0", name="pc0")
            w8 = cny.tile([128, 8, 128], F32, tag="fx2w", name="fx2w", bufs=1)
            nc.sync.dma_start(out=w8[:], in_=p.fc2_xt_w[:].rearrange(
                "(j p) n -> p j n", p=128))
            for j in range(8):
                nc.tensor.matmul(ps[:], xt1T[j][:], w8[:, j, :], start=(j == 0),
                                 stop=(j == 7))
            p.xt2 = p.head_pool.tile([GPC, 128], F32, tag="xt2", name="xt2")
            bb2 = cn.tile([GPC, 128], F32, tag="fxbb2", name="fxbb2", bufs=1)
            nc.sync.dma_start(out=bb2[:], in_=p.fc2_xt_b[:])
            nc.vector.tensor_tensor(out=p.xt2[:], in0=ps[:], in1=bb2[:], op=OP.add)
            t = tap("xt2", [GPC, 128])
            if t is not None:
                nc.sync.dma_start(out=t[:], in_=p.xt2[:])
        # whole CNN branch done -- release all its pools (LIFO: cny, cnw, cn)
        while st["cms"]:
            st["cms"].pop(0).__exit__(None, None, None)

    return {"stage1_open": stage1_open, "stage1_grp": stage1_grp,
            "stage1_close": stage1_close, "stage2": stage2, "stage3": stage3,
            "cleanup_cm": _Cleanup()}


# ---------------- fusion ----------------
def _fusion(p, tap):
    nc, tc = p.nc, p.tc
    _graph_head(p, tap)
    with (
        tc.tile_pool(name="fu", bufs=2) as fu,
        tc.tile_pool(name="fup", bufs=2, space="PSUM") as fup,
    ):
        xcT = []
        for src_ in (p.g2, p.xt2):
            t = fu.tile([128, GPC], F32, tag=f"xcT{len(xcT)}", name=f"xcT{len(xcT)}")
            _dve_T(nc, t, src_[:], 128)
            xcT.append(t)
        c1 = fu.tile([GPC, 1024], F32, tag="c1", name="c1")
        w2 = fu.tile([128, 2, 1024], F32, tag="f1w", name="f1w")
        nc.sync.dma_start(out=w2[:], in_=p.fc1_w[:].rearrange(
            "(j p) n -> p j n", p=128))
        for n in range(2):
            ps = fup.tile([GPC, 512], F32, tag="mm", name="mm")
            for j in range(2):
                nc.tensor.matmul(ps[:], xcT[j][:],
                                 w2[:, j, n * 512:(n + 1) * 512],
                                 start=(j == 0), stop=(j == 1))
            nc.vector.tensor_copy(out=c1[:, n * 512:(n + 1) * 512], in_=ps[:])
        bb = fu.tile([GPC, 1024], F32, tag="fbb", name="fbb")
        nc.sync.dma_start(out=bb[:], in_=p.fc1_b[:])
        nc.vector.tensor_tensor(out=c1[:], in0=c1[:], in1=bb[:], op=OP.add)
        c1b = fu.tile([GPC, 1024], F32, tag="c1b", name="c1b")
        nc.scalar.activation(c1b[:], c1[:], ACT.Relu)
        c1T = [fu.tile([128, GPC], F32, tag=f"c1T{j}", name=f"c1T{j}") for j in range(8)]
        for j in range(8):
            _dve_T(nc, c1T[j], c1b[:, j * 128:(j + 1) * 128], 128)
        ps = fup.tile([GPC, 256], F32, tag="mm", name="mm")
        wf2 = fu.tile([128, 8, 256], F32, tag="f2w", name="f2w")
        nc.sync.dma_start(out=wf2[:], in_=p.fc2_w[:].rearrange(
            "(j p) n -> p j n", p=128))
        for j in range(8):
            nc.tensor.matmul(ps[:], c1T[j][:], wf2[:, j, :], start=(j == 0),
                             stop=(j == 7))
        c2 = fu.tile([GPC, 256], F32, tag="c2", name="c2")
        bb2 = fu.tile([GPC, 256], F32, tag="fbb2", name="fbb2")
        nc.sync.dma_start(out=bb2[:], in_=p.fc2_b[:])
        nc.vector.tensor_tensor(out=c2[:], in0=ps[:], in1=bb2[:], op=OP.add)
        c2b = fu.tile([GPC, 256], F32, tag="c2b", name="c2b")
        nc.scalar.activation(c2b[:], c2[:], ACT.Relu)
        c2T = []
        for j in range(2):
            t = fu.tile([128, GPC], F32, tag=f"c2T{j}", name=f"c2T{j}")
            _dve_T(nc, t, c2b[:, j * 128:(j + 1) * 128], 128)
            c2T.append(t)
        ow = fu.tile([128, 2], F32, tag="ow", name="ow")
        for j in range(2):
            nc.sync.dma_start(out=ow[:, j:j + 1], in_=p.out_w[j * 128:(j + 1) * 128, :])
        ps = fup.tile([GPC, 1], F32, tag="mm", name="mm")
        for j in range(2):
            nc.tensor.matmul(ps[:], c2T[j][:], ow[:, j:j + 1],
                             start=(j == 0), stop=(j == 1))
        o = fu.tile([GPC, 1], F32, tag="o", name="o")
        nc.vector.tensor_copy(out=o[:], in_=ps[:])
        nc.sync.dma_start(out=p.out[:], in_=o[:])


# ------------------------------------------------------------------ entry
def _build_and_run(inputs, taps=()):
    T_blocks, in_maps, out_b = _host_prep(inputs)
    nc, p = build_program(T_blocks, taps=taps)
    res = run_bass_kernel_spmd(nc, in_maps, list(range(NCORES)))
    return res, out_b, p


def kernel(**inputs) -> np.ndarray:
    res, out_b, _ = _build_and_run(inputs)
    out = np.concatenate([res.results[c]["out"] for c in range(NCORES)], axis=0)
    return (out + out_b).astype(np.float32)
